# revision 1
# baseline (speedup 1.0000x reference)
"""GroupedQueryAttention Trainium2 kernel (8 NeuronCores, SPMD).

Sharding: core c -> (batch b = c // 4, kv-group g = c % 4).
Each core computes q/k/v projections for its 4 query heads + 1 kv head,
partial-RoPE, causal attention, and a partial out-projection over its
512 o-features; a 4-core ReduceScatter sums partials and scatters
S/4-row slices, which the host reassembles.

All device matmuls run in bf16 (fp32 PSUM accumulation). The host
pre-transposes the operands so the contraction dim lands on SBUF
partitions everywhere with no on-device transposes:
  xt   = x[b].T                  [D, S]
  wqt  = perm(wq)[group].T       [D, 512]   (rows RoPE-deinterleaved)
  wkt  = perm(wk)[group].T       [D, 128]
  wvt  = wv[group].T             [D, 128]
  wot  = wo[:, group_cols].T     [512, D]
The RoPE deinterleave permutation reorders each head's first 64 dims to
[evens, odds]; since q and k use the same permutation, q.k dot products
are unchanged and it never needs undoing.
"""

import math
import sys

sys.path.insert(0, "/opt/trn_rl_repo")

import numpy as np  # noqa: E402

D_MODEL = 2048
N_HEADS = 16
N_KV = 4
HEAD_DIM = 128
ROPE = 64
THETA = 10000.0
B = 2
HG = N_HEADS // N_KV  # 4 query heads per kv group
GD = HG * HEAD_DIM  # 512 o-features per group
N_CORES = 8
GROUPS = [[0, 1, 2, 3], [4, 5, 6, 7]]

_BUILD_CACHE: dict = {}


def build_kernel(S: int):
    """Build the per-core Bass program for sequence length S (multiple of 512)."""
    import concourse.bass as bass
    import concourse.mybir as mybir
    import concourse.tile as tile
    from concourse import bacc

    assert S % 512 == 0
    P = 128
    QT = 512  # q tile (free dim of scoresT)
    NJ = S // QT  # q tiles
    NO = D_MODEL // P  # contraction chunks for projections (16)
    NS = S // P  # seq chunks of 128
    bf16 = mybir.dt.bfloat16
    f32 = mybir.dt.float32
    scale = 1.0 / math.sqrt(HEAD_DIM)

    nc = bacc.Bacc(None, target_bir_lowering=False, debug=False, num_devices=N_CORES)

    xt_d = nc.declare_dram_parameter("xt", [D_MODEL, S], bf16, isOutput=False)
    wqt_d = nc.declare_dram_parameter("wqt", [D_MODEL, GD], bf16, isOutput=False)
    wkt_d = nc.declare_dram_parameter("wkt", [D_MODEL, HEAD_DIM], bf16, isOutput=False)
    wvt_d = nc.declare_dram_parameter("wvt", [D_MODEL, HEAD_DIM], bf16, isOutput=False)
    wot_d = nc.declare_dram_parameter("wot", [GD, D_MODEL], bf16, isOutput=False)
    cos_d = nc.declare_dram_parameter("cos", [ROPE, S], bf16, isOutput=False)
    sin_d = nc.declare_dram_parameter("sin", [ROPE, S], bf16, isOutput=False)
    msk_d = nc.declare_dram_parameter("masks", [4, P, QT], bf16, isOutput=False)
    out_d = nc.declare_dram_parameter("out", [S // 4, D_MODEL], f32, isOutput=True)

    with tile.TileContext(nc) as tc:
        with (
            tc.tile_pool(name="persist", bufs=1) as persist,
            tc.tile_pool(name="dram", bufs=1, space="DRAM") as dram,
        ):
            # ---- persistent SBUF state ----
            q_sb = persist.tile([P, HG, S], bf16)  # qT, per-head chunks
            k_sb = persist.tile([P, S], bf16)  # kT
            v_sb = persist.tile([P, NS, HEAD_DIM], bf16)  # v natural
            o_sb = persist.tile([P, HG, S], bf16)  # oT (normalized)
            cos_sb = persist.tile([ROPE, S], bf16)
            sin_sb = persist.tile([ROPE, S], bf16)
            msk_sb = persist.tile([P, 4, QT], bf16)
            ones_sb = persist.tile([P, P], bf16)
            wot_sb = persist.tile([P, HG, D_MODEL], bf16)

            partial_drams = [
                dram.tile([QT, D_MODEL], bf16, name=f"partial{i}") for i in range(NJ)
            ]
            vt_dram = dram.tile([HEAD_DIM, S], bf16)
            rs_drams = [
                dram.tile([QT // 4, D_MODEL], bf16, name=f"rs{i}") for i in range(NJ)
            ]

            nc.sync.dma_start(cos_sb[:], cos_d[:])
            nc.sync.dma_start(sin_sb[:], sin_d[:])
            nc.sync.dma_start(msk_sb[:], msk_d.rearrange("r p q -> p r q"))
            nc.vector.memset(ones_sb[:], 1.0)

            # ---- phase 1: projections (+RoPE) ----
            with (
                tc.tile_pool(name="proj_sb", bufs=1) as proj_sb,
                tc.tile_pool(name="proj_ps", bufs=4, space="PSUM") as proj_ps,
                tc.tile_pool(name="rope_tmp", bufs=2) as rtmp,
            ):
                xt_sb = proj_sb.tile([P, NO, S], bf16)
                wqt_sb = proj_sb.tile([P, NO, GD], bf16)
                wkt_sb = proj_sb.tile([P, NO, HEAD_DIM], bf16)
                wvt_sb = proj_sb.tile([P, NO, HEAD_DIM], bf16)
                vt_sb = proj_sb.tile([P, S], bf16)

                # chunked loads: each [128, 1, ...] slice is its own DMA so the
                # first projection matmuls start as soon as chunk 0 lands.
                xt_r = xt_d.rearrange("(o p) s -> p o s", p=P)
                wqt_r = wqt_d.rearrange("(o p) m -> p o m", p=P)
                wkt_r = wkt_d.rearrange("(o p) m -> p o m", p=P)
                wvt_r = wvt_d.rearrange("(o p) m -> p o m", p=P)
                for o in range(NO):
                    nc.sync.dma_start(wkt_sb[:, o, :], wkt_r[:, o, :])
                    nc.sync.dma_start(xt_sb[:, o, :], xt_r[:, o, :])
                    nc.sync.dma_start(wqt_sb[:, o, :], wqt_r[:, o, :])
                    nc.sync.dma_start(wvt_sb[:, o, :], wvt_r[:, o, :])
                nc.sync.dma_start(wot_sb[:], wot_d.rearrange("(o p) m -> p o m", p=P))

                def rope(dst, jsl):
                    # rotate-half form on deinterleaved rows:
                    #   rows 0:32 = a (even dims), 32:64 = b (odd dims)
                    #   new[0:64] = old[0:64]*cos64 + swap(old[0:64])*sin64
                    # with cos64 = [cosT; cosT], sin64 = [-sinT; sinT].
                    # All two-input DVE ops keep base partition 0 (HW rule).
                    xs = rtmp.tile([64, QT], bf16, tag="xs")
                    nc.vector.tensor_copy(xs[0:32, :], dst[32:64, jsl])
                    nc.vector.tensor_copy(xs[32:64, :], dst[0:32, jsl])
                    t = rtmp.tile([64, QT], bf16, tag="t")
                    u = rtmp.tile([64, QT], bf16, tag="u")
                    nc.vector.tensor_mul(t[:], xs[:], sin_sb[:, jsl])
                    nc.vector.tensor_mul(u[:], dst[0:64, jsl], cos_sb[:, jsl])
                    nc.vector.tensor_add(dst[0:64, jsl], u[:], t[:])

                # kT = wkt.T @ xt  -> [128 dk, S]
                for j in range(NJ):
                    jsl = bass.ts(j, QT)
                    ps = proj_ps.tile([P, QT], f32, tag="ps")
                    for o in range(NO):
                        nc.tensor.matmul(
                            ps[:],
                            wkt_sb[:, o, :],
                            xt_sb[:, o, jsl],
                            start=(o == 0),
                            stop=(o == NO - 1),
                        )
                    nc.vector.tensor_copy(k_sb[:, jsl], ps[:])
                    rope(k_sb, jsl)

                # qT per head -> [128 dq, S] x4
                for h in range(HG):
                    for j in range(NJ):
                        jsl = bass.ts(j, QT)
                        ps = proj_ps.tile([P, QT], f32, tag="ps")
                        for o in range(NO):
                            nc.tensor.matmul(
                                ps[:],
                                wqt_sb[:, o, bass.ts(h, P)],
                                xt_sb[:, o, jsl],
                                start=(o == 0),
                                stop=(o == NO - 1),
                            )
                        nc.vector.tensor_copy(q_sb[:, h, jsl], ps[:])
                        rope(q_sb[:, h, :], jsl)

                # vT = wvt.T @ xt -> [128 dv, S]; then DMA-transpose to v natural
                for j in range(NJ):
                    jsl = bass.ts(j, QT)
                    ps = proj_ps.tile([P, QT], f32, tag="ps")
                    for o in range(NO):
                        nc.tensor.matmul(
                            ps[:],
                            wvt_sb[:, o, :],
                            xt_sb[:, o, jsl],
                            start=(o == 0),
                            stop=(o == NO - 1),
                        )
                    nc.vector.tensor_copy(vt_sb[:, jsl], ps[:])
                nc.sync.dma_start(vt_dram[:], vt_sb[:])
                for o in range(NS):
                    nc.sync.dma_start_transpose(
                        v_sb[:, o, :], vt_dram[:, bass.ts(o, P)]
                    )

            # ---- phase 2: attention ----
            with (
                tc.tile_pool(name="attn_sb", bufs=3) as attn_sb,
                tc.tile_pool(name="ps_sc", bufs=2, space="PSUM") as ps_sc,
                tc.tile_pool(name="ps_acc", bufs=2, space="PSUM") as ps_acc,
                tc.tile_pool(name="ps_sum", bufs=2, space="PSUM") as ps_sum,
            ):
                for h in range(HG):
                    for j in range(NJ):
                        jsl = bass.ts(j, QT)
                        nk = 4 * (j + 1)  # causal: k chunks 0..nk-1
                        po = ps_acc.tile([P, QT], f32, tag="po")
                        psum = ps_sum.tile([P, QT], f32, tag="psum")
                        for c2 in range(nk // 2):
                            sc = ps_sc.tile([P, 2 * QT], f32, tag="sc")
                            pt = attn_sb.tile([P, 2 * QT], bf16, tag="pt")
                            for t in (0, 1):
                                c = 2 * c2 + t
                                nc.tensor.matmul(
                                    sc[:, bass.ts(t, QT)],
                                    k_sb[:, bass.ts(c, P)],
                                    q_sb[:, h, jsl],
                                    start=True,
                                    stop=True,
                                )
                            nc.scalar.activation(
                                pt[:],
                                sc[:],
                                mybir.ActivationFunctionType.Exp,
                                scale=scale,
                            )
                            for t in (0, 1):
                                c = 2 * c2 + t
                                r = c - 4 * j
                                if 0 <= r < 4:
                                    nc.vector.tensor_mul(
                                        pt[:, bass.ts(t, QT)],
                                        pt[:, bass.ts(t, QT)],
                                        msk_sb[:, r, :],
                                    )
                            for t in (0, 1):
                                c = 2 * c2 + t
                                nc.tensor.matmul(
                                    po[:],
                                    v_sb[:, c, :],
                                    pt[:, bass.ts(t, QT)],
                                    start=(c == 0),
                                    stop=(c == nk - 1),
                                )
                                nc.tensor.matmul(
                                    psum[:],
                                    ones_sb[:],
                                    pt[:, bass.ts(t, QT)],
                                    start=(c == 0),
                                    stop=(c == nk - 1),
                                )
                        rcp = attn_sb.tile([P, QT], f32, tag="rcp")
                        nc.vector.reciprocal_approx_fast(rcp[:], psum[:])
                        nc.vector.tensor_mul(o_sb[:, h, jsl], po[:], rcp[:])

            # ---- phase 3: out-projection (partial) + overlapped reduce-scatter ----
            # Row-block i (rows 512i:512i+512) is reduce-scattered as soon as
            # its partials are written, overlapping later out-proj blocks;
            # block i's scatter hands this core rows 512i+128g:+128 (host
            # reassembles that mapping).
            with (
                tc.tile_pool(name="oproj_sb", bufs=3) as oproj_sb,
                tc.tile_pool(name="ps_out", bufs=4, space="PSUM") as ps_out,
                tc.tile_pool(name="fin", bufs=2) as fin,
            ):
                for blk in range(NJ):
                    for mm in range(QT // P):
                        m = blk * (QT // P) + mm
                        for n2 in range(D_MODEL // QT):
                            ps = ps_out.tile([P, QT], f32, tag="ps")
                            for h in range(HG):
                                nc.tensor.matmul(
                                    ps[:],
                                    o_sb[:, h, bass.ts(m, P)],
                                    wot_sb[:, h, bass.ts(n2, QT)],
                                    start=(h == 0),
                                    stop=(h == HG - 1),
                                )
                            st = oproj_sb.tile([P, QT], bf16, tag="st")
                            nc.scalar.copy(st[:], ps[:])
                            nc.sync.dma_start(
                                partial_drams[blk][bass.ts(mm, P), bass.ts(n2, QT)],
                                st[:],
                            )
                    nc.gpsimd.collective_compute(
                        "ReduceScatter",
                        mybir.AluOpType.add,
                        replica_groups=GROUPS,
                        ins=[partial_drams[blk].opt()],
                        outs=[rs_drams[blk].opt()],
                    )
                    t16 = fin.tile([P, D_MODEL], bf16, tag="t16")
                    t32 = fin.tile([P, D_MODEL], f32, tag="t32")
                    nc.sync.dma_start(t16[:], rs_drams[blk][:])
                    nc.vector.tensor_copy(t32[:], t16[:])
                    nc.sync.dma_start(out_d[bass.ts(blk, P), :], t32[:])

    nc.compile()
    return nc


def host_prep(x, wq, wk, wv, wo, S):
    """Build the 8 per-core input maps (numpy, bf16)."""
    import ml_dtypes

    bf = ml_dtypes.bfloat16
    perm = np.concatenate(
        [np.arange(0, ROPE, 2), np.arange(1, ROPE, 2), np.arange(ROPE, HEAD_DIM)]
    )
    wq_p = wq.reshape(N_HEADS, HEAD_DIM, D_MODEL)[:, perm, :]
    wk_p = wk.reshape(N_KV, HEAD_DIM, D_MODEL)[:, perm, :]

    inv = THETA ** (-np.arange(0, ROPE, 2, dtype=np.float64) / ROPE)  # [32]
    t = np.arange(S, dtype=np.float64)
    ang = np.outer(inv, t)  # [32, S]
    cosT, sinT = np.cos(ang), np.sin(ang)
    cos = np.ascontiguousarray(np.concatenate([cosT, cosT], 0)).astype(bf)  # [64,S]
    sin = np.ascontiguousarray(np.concatenate([-sinT, sinT], 0)).astype(bf)  # [64,S]

    # causal masks for diagonal tiles: r = k_chunk - 4*j in [0,4)
    kk = np.arange(128)[:, None]
    qq = np.arange(512)[None, :]
    masks = np.stack(
        [(128 * r + kk <= qq) for r in range(4)]
    ).astype(bf)  # [4,128,512]

    in_maps = []
    for c in range(N_CORES):
        b, g = divmod(c, 4)
        xt = np.ascontiguousarray(x[b, :S].T).astype(bf)
        wqt = np.ascontiguousarray(
            wq_p[HG * g : HG * (g + 1)].reshape(GD, D_MODEL).T
        ).astype(bf)
        wkt = np.ascontiguousarray(wk_p[g].T).astype(bf)
        wvt = np.ascontiguousarray(wv[HEAD_DIM * g : HEAD_DIM * (g + 1)].T).astype(bf)
        wot = np.ascontiguousarray(wo[:, GD * g : GD * (g + 1)].T).astype(bf)
        in_maps.append(
            {
                "xt": xt,
                "wqt": wqt,
                "wkt": wkt,
                "wvt": wvt,
                "wot": wot,
                "cos": cos,
                "sin": sin,
                "masks": masks,
            }
        )
    return in_maps


def run(x, wq, wk, wv, wo, S=None, trace=False):
    from concourse.bass_utils import run_bass_kernel_spmd

    if S is None:
        S = x.shape[1]
    if S not in _BUILD_CACHE:
        _BUILD_CACHE[S] = build_kernel(S)
    nc = _BUILD_CACHE[S]
    in_maps = host_prep(x, wq, wk, wv, wo, S)
    res = run_bass_kernel_spmd(nc, in_maps, core_ids=list(range(N_CORES)), trace=trace)
    out = np.empty((B, S, D_MODEL), np.float32)
    nj = S // 512
    for c in range(N_CORES):
        b, g = divmod(c, 4)
        o = res.results[c]["out"]  # [nj*128, D]: row blk*128+r = global 512*blk+128*g+r
        for blk in range(nj):
            out[b, 512 * blk + 128 * g : 512 * blk + 128 * (g + 1), :] = o[
                128 * blk : 128 * (blk + 1)
            ]
    return out, res


def kernel(x, wq, wk, wv, wo):
    x = np.asarray(x, np.float32)
    wq = np.asarray(wq, np.float32)
    wk = np.asarray(wk, np.float32)
    wv = np.asarray(wv, np.float32)
    wo = np.asarray(wo, np.float32)
    out, _ = run(x, wq, wk, wv, wo)
    return out



# revision 6
# speedup vs baseline: 1.3182x; 1.3182x over previous
"""GroupedQueryAttention Trainium2 kernel (8 NeuronCores, SPMD).

Sharding: core c -> (batch b = c // 4, kv-group g = c % 4).
Each core computes q/k/v projections for its 4 query heads + 1 kv head,
partial-RoPE, and causal attention for its heads over the full sequence.
The attention outputs are then resharded with a single 8-way AllToAll
(fired per 512-row seq block, overlapped with later attention blocks):
core d ends up owning seq rows [512j+64d, 512j+64d+64) of BOTH batches
for every block j, with all 16 heads' features. Each core then runs the
full out-projection for its 512 rows (full wo, no reduction needed).

This replaces the baseline's 4-rank ReduceScatter of 8MB fp-partials
(~32 GB/s -> ~250us unoverlapped tail) with a 2MB AllToAll on the fast
copy path, overlapped behind attention.

All device matmuls run in bf16 (fp32 PSUM accumulation). The host
pre-transposes operands so the contraction dim lands on SBUF partitions
with no on-device transposes (except V, transposed on the PE array):
  xt   = x[b].T                  [D, S]
  wqt  = perm(wq)[group].T       [D, 512]   (rows RoPE-deinterleaved)
  wkt  = perm(wk)[group].T       [D, 128]
  wvt  = wv[group].T             [D, 128]
  wot  = wo.T                    [D, D]     (full, same on every core)
The RoPE deinterleave permutation reorders each head's first 64 dims to
[evens, odds]; since q and k use the same permutation, q.k dot products
are unchanged and it never needs undoing.
"""

import math
import sys

sys.path.insert(0, "/opt/trn_rl_repo")

import numpy as np  # noqa: E402

D_MODEL = 2048
N_HEADS = 16
N_KV = 4
HEAD_DIM = 128
ROPE = 64
THETA = 10000.0
B = 2
HG = N_HEADS // N_KV  # 4 query heads per kv group
GD = HG * HEAD_DIM  # 512 o-features per group
N_CORES = 8
ALL_CORES = [list(range(N_CORES))]

_BUILD_CACHE: dict = {}


def build_kernel(S: int):
    """Build the per-core Bass program for sequence length S (multiple of 512)."""
    import concourse.bass as bass
    import concourse.mybir as mybir
    import concourse.tile as tile
    from concourse import bacc

    assert S % 512 == 0
    P = 128
    QT = 512  # q tile (free dim of scoresT)
    NJ = S // QT  # q tiles / seq blocks
    NO = D_MODEL // P  # contraction chunks for projections (16)
    NS = S // P  # seq chunks of 128
    RB = QT // N_CORES  # rows per (block, dest core, batch) = 64
    bf16 = mybir.dt.bfloat16
    f32 = mybir.dt.float32
    scale = 1.0 / math.sqrt(HEAD_DIM)

    nc = bacc.Bacc(None, target_bir_lowering=False, debug=False, num_devices=N_CORES)

    xt_d = nc.declare_dram_parameter("xt", [D_MODEL, S], bf16, isOutput=False)
    wqt_d = nc.declare_dram_parameter("wqt", [D_MODEL, GD], bf16, isOutput=False)
    wkt_d = nc.declare_dram_parameter("wkt", [D_MODEL, HEAD_DIM], bf16, isOutput=False)
    wvt_d = nc.declare_dram_parameter("wvt", [D_MODEL, HEAD_DIM], bf16, isOutput=False)
    wot_d = nc.declare_dram_parameter("wot", [D_MODEL, D_MODEL], bf16, isOutput=False)
    cos_d = nc.declare_dram_parameter("cos", [ROPE, S], bf16, isOutput=False)
    sin_d = nc.declare_dram_parameter("sin", [ROPE, S], bf16, isOutput=False)
    msk_d = nc.declare_dram_parameter("masks", [4, P, QT], bf16, isOutput=False)
    idn_d = nc.declare_dram_parameter("ident", [P, P], bf16, isOutput=False)
    # per-core output rows: row 128*j + 64*b + r = (batch b, seq 512*j + 64*core + r)
    out_d = nc.declare_dram_parameter("out", [S // 4, D_MODEL], f32, isOutput=True)

    with tile.TileContext(nc) as tc:
        with (
            tc.tile_pool(name="persist", bufs=1) as persist,
            tc.tile_pool(name="dram", bufs=1, space="DRAM") as dram,
        ):
            # ---- persistent SBUF state ----
            q_sb = persist.tile([P, HG, S], bf16)  # qT, per-head chunks
            k_sb = persist.tile([P, S], bf16)  # kT
            v_sb = persist.tile([P, NS, HEAD_DIM], bf16)  # v natural
            cos_sb = persist.tile([ROPE, S], bf16)
            sin_sb = persist.tile([ROPE, S], bf16)
            msk_sb = persist.tile([P, 4, QT], bf16)
            ones_sb = persist.tile([P, P], bf16)
            idn_sb = persist.tile([P, P], bf16)
            wot_sb = persist.tile([P, NO, D_MODEL], bf16)  # full wo.T, chunked

            a2a_in = [
                dram.tile([N_CORES, GD, RB], bf16, name=f"a2ain{j}") for j in range(NJ)
            ]
            a2a_out = [
                dram.tile([N_CORES, GD, RB], bf16, name=f"a2aout{j}") for j in range(NJ)
            ]

            nc.sync.dma_start(cos_sb[:], cos_d[:])
            nc.sync.dma_start(sin_sb[:], sin_d[:])
            nc.sync.dma_start(msk_sb[:], msk_d.rearrange("r p q -> p r q"))
            nc.sync.dma_start(idn_sb[:], idn_d[:])
            nc.vector.memset(ones_sb[:], 1.0)

            # ---- phase 1: projections (+RoPE) ----
            with (
                tc.tile_pool(name="proj_sb", bufs=1) as proj_sb,
                tc.tile_pool(name="proj_ps", bufs=4, space="PSUM") as proj_ps,
                tc.tile_pool(name="vt_ps", bufs=2, space="PSUM") as vt_ps,
                tc.tile_pool(name="rope_tmp", bufs=2) as rtmp,
            ):
                xt_sb = proj_sb.tile([P, NO, S], bf16)
                wqt_sb = proj_sb.tile([P, NO, GD], bf16)
                wkt_sb = proj_sb.tile([P, NO, HEAD_DIM], bf16)
                wvt_sb = proj_sb.tile([P, NO, HEAD_DIM], bf16)
                vt_sb = proj_sb.tile([P, S], bf16)

                # j-blocked loads: K proj of block 0 can start once wk + the
                # 2MB xt block 0 land, ~7us in, instead of waiting for all 8MB.
                xt_r = xt_d.rearrange("(o p) s -> p o s", p=P)
                wqt_r = wqt_d.rearrange("(o p) m -> p o m", p=P)
                wkt_r = wkt_d.rearrange("(o p) m -> p o m", p=P)
                wvt_r = wvt_d.rearrange("(o p) m -> p o m", p=P)
                for o in range(NO):
                    nc.sync.dma_start(wkt_sb[:, o, :], wkt_r[:, o, :])
                    nc.sync.dma_start(wvt_sb[:, o, :], wvt_r[:, o, :])
                for j in range(NJ):
                    jsl = bass.ts(j, QT)
                    for o in range(NO):
                        nc.sync.dma_start(xt_sb[:, o, jsl], xt_r[:, o, jsl])
                for o in range(NO):
                    nc.sync.dma_start(wqt_sb[:, o, :], wqt_r[:, o, :])
                nc.sync.dma_start(wot_sb[:], wot_d.rearrange("(o p) m -> p o m", p=P))

                def rope(dst, jsl):
                    # rotate-half form on deinterleaved rows:
                    #   rows 0:32 = a (even dims), 32:64 = b (odd dims)
                    #   new[0:64] = old[0:64]*cos64 + swap(old[0:64])*sin64
                    # with cos64 = [cosT; cosT], sin64 = [-sinT; sinT].
                    # All two-input DVE ops keep base partition 0 (HW rule).
                    xs = rtmp.tile([64, QT], bf16, tag="xs")
                    nc.vector.tensor_copy(xs[0:32, :], dst[32:64, jsl])
                    nc.vector.tensor_copy(xs[32:64, :], dst[0:32, jsl])
                    t = rtmp.tile([64, QT], bf16, tag="t")
                    u = rtmp.tile([64, QT], bf16, tag="u")
                    nc.vector.tensor_mul(t[:], xs[:], sin_sb[:, jsl])
                    nc.vector.tensor_mul(u[:], dst[0:64, jsl], cos_sb[:, jsl])
                    nc.vector.tensor_add(dst[0:64, jsl], u[:], t[:])

                # kT = wkt.T @ xt  -> [128 dk, S]
                for j in range(NJ):
                    jsl = bass.ts(j, QT)
                    ps = proj_ps.tile([P, QT], f32, tag="ps")
                    for o in range(NO):
                        nc.tensor.matmul(
                            ps[:],
                            wkt_sb[:, o, :],
                            xt_sb[:, o, jsl],
                            start=(o == 0),
                            stop=(o == NO - 1),
                        )
                    nc.vector.tensor_copy(k_sb[:, jsl], ps[:])
                    rope(k_sb, jsl)

                # vT = wvt.T @ xt -> [128 dv, S]; then PE-transpose to v natural
                for j in range(NJ):
                    jsl = bass.ts(j, QT)
                    ps = proj_ps.tile([P, QT], f32, tag="ps")
                    for o in range(NO):
                        nc.tensor.matmul(
                            ps[:],
                            wvt_sb[:, o, :],
                            xt_sb[:, o, jsl],
                            start=(o == 0),
                            stop=(o == NO - 1),
                        )
                    nc.vector.tensor_copy(vt_sb[:, jsl], ps[:])
                    for cc in range(QT // P):
                        c = j * (QT // P) + cc
                        tp = vt_ps.tile([P, P], bf16, tag="vtp")
                        nc.tensor.transpose(tp[:], vt_sb[:, bass.ts(c, P)], idn_sb[:])
                        nc.scalar.copy(v_sb[:, c, :], tp[:])

                # qT per head -> [128 dq, S] x4
                for h in range(HG):
                    for j in range(NJ):
                        jsl = bass.ts(j, QT)
                        ps = proj_ps.tile([P, QT], f32, tag="ps")
                        for o in range(NO):
                            nc.tensor.matmul(
                                ps[:],
                                wqt_sb[:, o, bass.ts(h, P)],
                                xt_sb[:, o, jsl],
                                start=(o == 0),
                                stop=(o == NO - 1),
                            )
                        nc.vector.tensor_copy(q_sb[:, h, jsl], ps[:])
                        rope(q_sb[:, h, :], jsl)

            # ---- phase 2: attention (j-outer) + per-block AllToAll +
            #      interleaved out-projection ----
            with (
                tc.tile_pool(name="attn_sb", bufs=3) as attn_sb,
                tc.tile_pool(name="osb", bufs=2) as osb_pool,
                tc.tile_pool(name="og", bufs=2) as og_pool,
                tc.tile_pool(name="fin", bufs=2) as fin,
                tc.tile_pool(name="ps_sc", bufs=2, space="PSUM") as ps_sc,
                tc.tile_pool(name="ps_acc", bufs=1, space="PSUM") as ps_acc,
                tc.tile_pool(name="ps_sum", bufs=1, space="PSUM") as ps_sum,
                tc.tile_pool(name="ps_out", bufs=2, space="PSUM") as ps_out,
            ):

                def outproj(j):
                    # out rows of block j: [128] = 64 batch-0 rows + 64 batch-1
                    # rows this core owns; full 2048-feature contraction.
                    og_sb = og_pool.tile([P, NO, P], bf16, tag="og")
                    # og[p, 4g+fi, 64b+r] = a2a_out[j][4b+g, 128fi+p, r]
                    # og[p, 4g+fi, 64b+r] = a2a_out[j][4b+g, 128fi+p, r]
                    src = a2a_out[j].rearrange("c (fi p) r -> c p fi r", fi=HG)
                    for b in range(B):
                        for g in range(HG):
                            nc.sync.dma_start(
                                og_sb[:, HG * g : HG * (g + 1), RB * b : RB * (b + 1)],
                                src[HG * b + g],
                            )
                    ot = fin.tile([P, D_MODEL], f32, tag="ot")
                    for n in range(D_MODEL // QT):
                        ps = ps_out.tile([P, QT], f32, tag="pout")
                        for c in range(NO):
                            nc.tensor.matmul(
                                ps[:],
                                og_sb[:, c, :],
                                wot_sb[:, c, bass.ts(n, QT)],
                                start=(c == 0),
                                stop=(c == NO - 1),
                            )
                        nc.scalar.copy(ot[:, bass.ts(n, QT)], ps[:])
                    nc.sync.dma_start(out_d[bass.ts(j, P), :], ot[:])

                for j in range(NJ):
                    jsl = bass.ts(j, QT)
                    nk = 4 * (j + 1)  # causal: k chunks 0..nk-1
                    o_sb = osb_pool.tile([P, HG, QT], bf16, tag="osb")
                    for h in range(HG):
                        po = ps_acc.tile([P, QT], f32, tag="po")
                        psum = ps_sum.tile([P, QT], f32, tag="psum")
                        for c2 in range(nk // 2):
                            sc = ps_sc.tile([P, 2 * QT], f32, tag="sc")
                            pt = attn_sb.tile([P, 2 * QT], bf16, tag="pt")
                            for t in (0, 1):
                                c = 2 * c2 + t
                                nc.tensor.matmul(
                                    sc[:, bass.ts(t, QT)],
                                    k_sb[:, bass.ts(c, P)],
                                    q_sb[:, h, jsl],
                                    start=True,
                                    stop=True,
                                )
                            nc.scalar.activation(
                                pt[:],
                                sc[:],
                                mybir.ActivationFunctionType.Exp,
                                scale=scale,
                            )
                            for t in (0, 1):
                                c = 2 * c2 + t
                                r = c - 4 * j
                                if 0 <= r < 4:
                                    nc.vector.tensor_mul(
                                        pt[:, bass.ts(t, QT)],
                                        pt[:, bass.ts(t, QT)],
                                        msk_sb[:, r, :],
                                    )
                            for t in (0, 1):
                                c = 2 * c2 + t
                                nc.tensor.matmul(
                                    po[:],
                                    v_sb[:, c, :],
                                    pt[:, bass.ts(t, QT)],
                                    start=(c == 0),
                                    stop=(c == nk - 1),
                                )
                                nc.tensor.matmul(
                                    psum[:],
                                    ones_sb[:],
                                    pt[:, bass.ts(t, QT)],
                                    start=(c == 0),
                                    stop=(c == nk - 1),
                                )
                        rcp = attn_sb.tile([P, QT], f32, tag="rcp")
                        nc.vector.reciprocal_approx_fast(rcp[:], psum[:])
                        nc.vector.tensor_mul(o_sb[:, h, :], po[:], rcp[:])

                    # ship block j: a2a_in[j][d, 128h+p, r] = o_sb[p, h, 64d+r]
                    in_v = a2a_in[j].rearrange("d (h p) r -> p h d r", h=HG)
                    for h in range(HG):
                        nc.sync.dma_start(
                            in_v[:, h],
                            o_sb[:, h, :].rearrange("p (d r) -> p d r", d=N_CORES),
                        )
                    nc.gpsimd.collective_compute(
                        "AllToAll",
                        mybir.AluOpType.bypass,
                        replica_groups=ALL_CORES,
                        ins=[a2a_in[j].opt()],
                        outs=[a2a_out[j].opt()],
                    )
                    # out-projection of the previous block overlaps this
                    # block's a2a; blocks 0..j-1's a2a overlapped attention.
                    if j > 0:
                        outproj(j - 1)
                outproj(NJ - 1)

    nc.compile()
    return nc


def host_prep(x, wq, wk, wv, wo, S):
    """Build the 8 per-core input maps (numpy, bf16)."""
    import ml_dtypes

    bf = ml_dtypes.bfloat16
    perm = np.concatenate(
        [np.arange(0, ROPE, 2), np.arange(1, ROPE, 2), np.arange(ROPE, HEAD_DIM)]
    )
    wq_p = wq.reshape(N_HEADS, HEAD_DIM, D_MODEL)[:, perm, :]
    wk_p = wk.reshape(N_KV, HEAD_DIM, D_MODEL)[:, perm, :]

    inv = THETA ** (-np.arange(0, ROPE, 2, dtype=np.float64) / ROPE)  # [32]
    t = np.arange(S, dtype=np.float64)
    ang = np.outer(inv, t)  # [32, S]
    cosT, sinT = np.cos(ang), np.sin(ang)
    cos = np.ascontiguousarray(np.concatenate([cosT, cosT], 0)).astype(bf)  # [64,S]
    sin = np.ascontiguousarray(np.concatenate([-sinT, sinT], 0)).astype(bf)  # [64,S]

    # causal masks for diagonal tiles: r = k_chunk - 4*j in [0,4)
    kk = np.arange(128)[:, None]
    qq = np.arange(512)[None, :]
    masks = np.stack(
        [(128 * r + kk <= qq) for r in range(4)]
    ).astype(bf)  # [4,128,512]

    ident = np.eye(128, dtype=np.float32).astype(bf)
    wot = np.ascontiguousarray(wo.T).astype(bf)  # [D in, D out]

    in_maps = []
    for c in range(N_CORES):
        b, g = divmod(c, 4)
        xt = np.ascontiguousarray(x[b, :S].T).astype(bf)
        wqt = np.ascontiguousarray(
            wq_p[HG * g : HG * (g + 1)].reshape(GD, D_MODEL).T
        ).astype(bf)
        wkt = np.ascontiguousarray(wk_p[g].T).astype(bf)
        wvt = np.ascontiguousarray(wv[HEAD_DIM * g : HEAD_DIM * (g + 1)].T).astype(bf)
        in_maps.append(
            {
                "xt": xt,
                "wqt": wqt,
                "wkt": wkt,
                "wvt": wvt,
                "wot": wot,
                "cos": cos,
                "sin": sin,
                "masks": masks,
                "ident": ident,
            }
        )
    return in_maps


def run(x, wq, wk, wv, wo, S=None, trace=False):
    from concourse.bass_utils import run_bass_kernel_spmd

    if S is None:
        S = x.shape[1]
    if S not in _BUILD_CACHE:
        _BUILD_CACHE[S] = build_kernel(S)
    nc = _BUILD_CACHE[S]
    in_maps = host_prep(x, wq, wk, wv, wo, S)
    res = run_bass_kernel_spmd(nc, in_maps, core_ids=list(range(N_CORES)), trace=trace)
    out = np.empty((B, S, D_MODEL), np.float32)
    nj = S // 512
    rb = 512 // N_CORES
    for d in range(N_CORES):
        o = res.results[d]["out"]  # [nj*128, D]
        for j in range(nj):
            for b in range(B):
                out[b, 512 * j + rb * d : 512 * j + rb * (d + 1), :] = o[
                    128 * j + rb * b : 128 * j + rb * (b + 1)
                ]
    return out, res


def kernel(x, wq, wk, wv, wo):
    x = np.asarray(x, np.float32)
    wq = np.asarray(wq, np.float32)
    wk = np.asarray(wk, np.float32)
    wv = np.asarray(wv, np.float32)
    wo = np.asarray(wo, np.float32)
    out, _ = run(x, wq, wk, wv, wo)
    return out


# revision 13
# speedup vs baseline: 1.4052x; 1.0660x over previous
"""GroupedQueryAttention Trainium2 kernel (8 NeuronCores, SPMD).

Sharding: core c -> (batch b = c // 4, kv-group g = c % 4).
Each core computes q/k/v projections for its 4 query heads + 1 kv head,
partial-RoPE, and causal attention for its heads over the full sequence.
The attention outputs are then resharded with a single 8-way AllToAll
(fired per 512-row seq block, overlapped with later attention blocks):
core d ends up owning seq rows [512j+64d, 512j+64d+64) of BOTH batches
for every block j, with all 16 heads' features. Each core then runs the
full out-projection for its 512 rows (full wo, no reduction needed).

This replaces the baseline's 4-rank ReduceScatter of 8MB fp-partials
(~32 GB/s -> ~250us unoverlapped tail) with a 2MB AllToAll on the fast
copy path, overlapped behind attention.

All device matmuls run in bf16 (fp32 PSUM accumulation). The host
pre-transposes operands so the contraction dim lands on SBUF partitions
with no on-device transposes (except V, transposed on the PE array):
  xt   = x[b].T                  [D, S]
  wqt  = perm(wq)[group].T       [D, 512]   (rows RoPE-deinterleaved)
  wkt  = perm(wk)[group].T       [D, 128]
  wvt  = wv[group].T             [D, 128]
  wot  = wo.T                    [D, D]     (full, same on every core)
The RoPE deinterleave permutation reorders each head's first 64 dims to
[evens, odds]; since q and k use the same permutation, q.k dot products
are unchanged and it never needs undoing.
"""

import math
import sys

sys.path.insert(0, "/opt/trn_rl_repo")

import numpy as np  # noqa: E402

D_MODEL = 2048
N_HEADS = 16
N_KV = 4
HEAD_DIM = 128
ROPE = 64
THETA = 10000.0
B = 2
HG = N_HEADS // N_KV  # 4 query heads per kv group
GD = HG * HEAD_DIM  # 512 o-features per group
N_CORES = 8
ALL_CORES = [list(range(N_CORES))]

_BUILD_CACHE: dict = {}


def build_kernel(S: int):
    """Build the per-core Bass program for sequence length S (multiple of 512)."""
    import concourse.bass as bass
    import concourse.mybir as mybir
    import concourse.tile as tile
    from concourse import bacc

    assert S % 512 == 0
    P = 128
    QT = 512  # q tile (free dim of scoresT)
    NJ = S // QT  # q tiles / seq blocks
    NO = D_MODEL // P  # contraction chunks for projections (16)
    NS = S // P  # seq chunks of 128
    RB = QT // N_CORES  # rows per (block, dest core, batch) = 64
    bf16 = mybir.dt.bfloat16
    f32 = mybir.dt.float32
    scale = 1.0 / math.sqrt(HEAD_DIM)

    nc = bacc.Bacc(None, target_bir_lowering=False, debug=False, num_devices=N_CORES)

    xt_d = nc.declare_dram_parameter("xt", [D_MODEL, S], bf16, isOutput=False)
    wqt_d = nc.declare_dram_parameter("wqt", [D_MODEL, GD], bf16, isOutput=False)
    wkt_d = nc.declare_dram_parameter("wkt", [D_MODEL, HEAD_DIM], bf16, isOutput=False)
    wvt_d = nc.declare_dram_parameter("wvt", [D_MODEL, HEAD_DIM], bf16, isOutput=False)
    wot_d = nc.declare_dram_parameter("wot", [D_MODEL, D_MODEL], bf16, isOutput=False)
    cos_d = nc.declare_dram_parameter("cos", [ROPE, S], bf16, isOutput=False)
    sin_d = nc.declare_dram_parameter("sin", [ROPE, S], bf16, isOutput=False)
    msk_d = nc.declare_dram_parameter("masks", [4, P, QT], bf16, isOutput=False)
    idn_d = nc.declare_dram_parameter("ident", [P, P], bf16, isOutput=False)
    # per-core output rows: row 128*j + 64*b + r = (batch b, seq 512*j + 64*core + r)
    out_d = nc.declare_dram_parameter("out", [S // 4, D_MODEL], f32, isOutput=True)

    with tile.TileContext(nc) as tc:
        with (
            tc.tile_pool(name="persist", bufs=1) as persist,
            tc.tile_pool(name="dram", bufs=1, space="DRAM") as dram,
        ):
            # ---- persistent SBUF state ----
            q_sb = persist.tile([P, HG, S], bf16)  # qT, per-head chunks
            k_sb = persist.tile([P, S], bf16)  # kT
            v_sb = persist.tile([P, NS, HEAD_DIM], bf16)  # v natural
            cos_sb = persist.tile([ROPE, S], bf16)
            sin_sb = persist.tile([ROPE, S], bf16)
            msk_sb = persist.tile([P, 4, QT], bf16)
            ones_sb = persist.tile([P, P], bf16)
            idn_sb = persist.tile([P, P], bf16)
            wot_sb = persist.tile([P, NO, D_MODEL], bf16)  # full wo.T, chunked

            a2a_in = [
                dram.tile([N_CORES * GD, RB], bf16, name=f"a2ain{j}")
                for j in range(NJ)
            ]
            a2a_out = [
                dram.tile([N_CORES * GD, RB], bf16, name=f"a2aout{j}")
                for j in range(NJ)
            ]

            nc.vector.memset(ones_sb[:], 1.0)

            # ---- phase 1: projections (+RoPE) ----
            with (
                tc.tile_pool(name="proj_sb", bufs=1) as proj_sb,
                tc.tile_pool(name="proj_ps", bufs=4, space="PSUM") as proj_ps,
                tc.tile_pool(name="vt_ps", bufs=2, space="PSUM") as vt_ps,
                tc.tile_pool(name="rope_tmp", bufs=2) as rtmp,
            ):
                xt_sb = proj_sb.tile([P, NO, S], bf16)
                wqt_sb = proj_sb.tile([P, NO, GD], bf16)
                wkt_sb = proj_sb.tile([P, NO, HEAD_DIM], bf16)
                wvt_sb = proj_sb.tile([P, NO, HEAD_DIM], bf16)
                vt_sb = proj_sb.tile([P, S], bf16)

                # Consolidated loads in dependency order: K proj of block 0
                # starts once wk + the 2MB xt block 0 land (~8us in). Big
                # single DMAs avoid the ~0.6us-per-dispatch serialization
                # that cost ~25us of startup with per-chunk loads.
                xt_r = xt_d.rearrange("(o p) s -> p o s", p=P)
                nc.sync.dma_start(wkt_sb[:], wkt_d.rearrange("(o p) m -> p o m", p=P))
                nc.sync.dma_start(xt_sb[:, :, bass.ts(0, QT)], xt_r[:, :, bass.ts(0, QT)])
                nc.sync.dma_start(cos_sb[:], cos_d[:])
                nc.sync.dma_start(sin_sb[:], sin_d[:])
                nc.sync.dma_start(msk_sb[:], msk_d.rearrange("r p q -> p r q"))
                nc.sync.dma_start(idn_sb[:], idn_d[:])
                nc.sync.dma_start(wvt_sb[:], wvt_d.rearrange("(o p) m -> p o m", p=P))
                for j in range(1, NJ):
                    jsl = bass.ts(j, QT)
                    nc.sync.dma_start(xt_sb[:, :, jsl], xt_r[:, :, jsl])
                nc.sync.dma_start(wqt_sb[:], wqt_d.rearrange("(o p) m -> p o m", p=P))
                nc.sync.dma_start(wot_sb[:], wot_d.rearrange("(o p) m -> p o m", p=P))

                def rope(dst, jsl):
                    # rotate-half form on deinterleaved rows:
                    #   rows 0:32 = a (even dims), 32:64 = b (odd dims)
                    #   new[0:64] = old[0:64]*cos64 + swap(old[0:64])*sin64
                    # with cos64 = [cosT; cosT], sin64 = [-sinT; sinT].
                    # All two-input DVE ops keep base partition 0 (HW rule).
                    xs = rtmp.tile([64, QT], bf16, tag="xs")
                    nc.vector.tensor_copy(xs[0:32, :], dst[32:64, jsl])
                    nc.vector.tensor_copy(xs[32:64, :], dst[0:32, jsl])
                    t = rtmp.tile([64, QT], bf16, tag="t")
                    u = rtmp.tile([64, QT], bf16, tag="u")
                    nc.vector.tensor_mul(t[:], xs[:], sin_sb[:, jsl])
                    nc.vector.tensor_mul(u[:], dst[0:64, jsl], cos_sb[:, jsl])
                    nc.vector.tensor_add(dst[0:64, jsl], u[:], t[:])

                # kT = wkt.T @ xt  -> [128 dk, S]
                for j in range(NJ):
                    jsl = bass.ts(j, QT)
                    ps = proj_ps.tile([P, QT], f32, tag="ps")
                    for o in range(NO):
                        nc.tensor.matmul(
                            ps[:],
                            wkt_sb[:, o, :],
                            xt_sb[:, o, jsl],
                            start=(o == 0),
                            stop=(o == NO - 1),
                        )
                    nc.vector.tensor_copy(k_sb[:, jsl], ps[:])
                    rope(k_sb, jsl)

                # vT = wvt.T @ xt -> [128 dv, S]; then PE-transpose to v natural
                for j in range(NJ):
                    jsl = bass.ts(j, QT)
                    ps = proj_ps.tile([P, QT], f32, tag="ps")
                    for o in range(NO):
                        nc.tensor.matmul(
                            ps[:],
                            wvt_sb[:, o, :],
                            xt_sb[:, o, jsl],
                            start=(o == 0),
                            stop=(o == NO - 1),
                        )
                    nc.vector.tensor_copy(vt_sb[:, jsl], ps[:])
                    for cc in range(QT // P):
                        c = j * (QT // P) + cc
                        tp = vt_ps.tile([P, P], bf16, tag="vtp")
                        nc.tensor.transpose(tp[:], vt_sb[:, bass.ts(c, P)], idn_sb[:])
                        nc.scalar.copy(v_sb[:, c, :], tp[:])

                # qT per head -> [128 dq, S] x4
                for h in range(HG):
                    for j in range(NJ):
                        jsl = bass.ts(j, QT)
                        ps = proj_ps.tile([P, QT], f32, tag="ps")
                        for o in range(NO):
                            nc.tensor.matmul(
                                ps[:],
                                wqt_sb[:, o, bass.ts(h, P)],
                                xt_sb[:, o, jsl],
                                start=(o == 0),
                                stop=(o == NO - 1),
                            )
                        nc.vector.tensor_copy(q_sb[:, h, jsl], ps[:])
                        rope(q_sb[:, h, :], jsl)

            # ---- phase 2: attention (j-outer) + per-block AllToAll +
            #      interleaved out-projection ----
            with (
                tc.tile_pool(name="attn_sb", bufs=3) as attn_sb,
                tc.tile_pool(name="osb", bufs=2) as osb_pool,
                tc.tile_pool(name="og", bufs=2) as og_pool,
                tc.tile_pool(name="fin", bufs=2) as fin,
                tc.tile_pool(name="ps_sc", bufs=2, space="PSUM") as ps_sc,
                tc.tile_pool(name="ps_acc", bufs=1, space="PSUM") as ps_acc,
                tc.tile_pool(name="ps_sum", bufs=1, space="PSUM") as ps_sum,
                tc.tile_pool(name="ps_out", bufs=2, space="PSUM") as ps_out,
            ):

                def outproj(j):
                    # out rows of block j: [128] = 64 batch-0 rows + 64 batch-1
                    # rows this core owns; full 2048-feature contraction.
                    og_sb = og_pool.tile([P, NO, P], bf16, tag="og")
                    # og[p, 4g+fi, 64b+r] = a2a_out[j][4b+g, 128fi+p, r]
                    # og[p, 4g+fi, 64b+r] = a2a_out[j][(4b+g)*512 + 128fi+p, r];
                    # one DMA per batch half (sources 4b..4b+3 merge with fi).
                    src = a2a_out[j].rearrange("(b gfi p) r -> b p gfi r", b=B, p=P)
                    for b in range(B):
                        nc.sync.dma_start(
                            og_sb[:, :, RB * b : RB * (b + 1)], src[b]
                        )
                    ot = fin.tile([P, D_MODEL], f32, tag="ot")
                    for n in range(D_MODEL // QT):
                        ps = ps_out.tile([P, QT], f32, tag="pout")
                        for c in range(NO):
                            nc.tensor.matmul(
                                ps[:],
                                og_sb[:, c, :],
                                wot_sb[:, c, bass.ts(n, QT)],
                                start=(c == 0),
                                stop=(c == NO - 1),
                            )
                        nc.scalar.copy(ot[:, bass.ts(n, QT)], ps[:])
                    nc.sync.dma_start(out_d[bass.ts(j, P), :], ot[:])

                for j in range(NJ):
                    jsl = bass.ts(j, QT)
                    nk = 4 * (j + 1)  # causal: k chunks 0..nk-1
                    o_sb = osb_pool.tile([P, HG, QT], bf16, tag="osb")
                    for h in range(HG):
                        po = ps_acc.tile([P, QT], f32, tag="po")
                        acc = attn_sb.tile([P, QT], bf16, tag="acc")
                        for c2 in range(nk // 2):
                            sc = ps_sc.tile([P, 2 * QT], f32, tag="sc")
                            pt = attn_sb.tile([P, 2 * QT], bf16, tag="pt")
                            for t in (0, 1):
                                c = 2 * c2 + t
                                nc.tensor.matmul(
                                    sc[:, bass.ts(t, QT)],
                                    k_sb[:, bass.ts(c, P)],
                                    q_sb[:, h, jsl],
                                    start=True,
                                    stop=True,
                                )
                            nc.scalar.activation(
                                pt[:],
                                sc[:],
                                mybir.ActivationFunctionType.Exp,
                                scale=scale,
                            )
                            for t in (0, 1):
                                c = 2 * c2 + t
                                r = c - 4 * j
                                if 0 <= r < 4:
                                    nc.vector.tensor_mul(
                                        pt[:, bass.ts(t, QT)],
                                        pt[:, bass.ts(t, QT)],
                                        msk_sb[:, r, :],
                                    )
                            # softmax-denominator partials accumulate on DVE
                            # (per k-partition); replaces a per-chunk
                            # ones-matmul on the tensor engine.
                            for t in (0, 1):
                                c = 2 * c2 + t
                                if c == 0:
                                    nc.vector.tensor_copy(
                                        acc[:], pt[:, bass.ts(t, QT)]
                                    )
                                else:
                                    nc.vector.tensor_add(
                                        acc[:], acc[:], pt[:, bass.ts(t, QT)]
                                    )
                            for t in (0, 1):
                                c = 2 * c2 + t
                                nc.tensor.matmul(
                                    po[:],
                                    v_sb[:, c, :],
                                    pt[:, bass.ts(t, QT)],
                                    start=(c == 0),
                                    stop=(c == nk - 1),
                                )
                        # partition-reduce acc + broadcast in one matmul
                        rs = ps_sum.tile([P, QT], f32, tag="rs")
                        nc.tensor.matmul(
                            rs[:], ones_sb[:], acc[:], start=True, stop=True
                        )
                        rcp = attn_sb.tile([P, QT], f32, tag="rcp")
                        nc.vector.reciprocal_approx_fast(rcp[:], rs[:])
                        nc.vector.tensor_mul(o_sb[:, h, :], po[:], rcp[:])

                    # ship block j: a2a_in[j][(d*4+h)*128+p, r] = o_sb[p, h, 64d+r]
                    in_v = a2a_in[j].rearrange("(d h p) r -> p h d r", h=HG, p=P)
                    for h in range(HG):
                        nc.sync.dma_start(
                            in_v[:, h],
                            o_sb[:, h, :].rearrange("p (d r) -> p d r", d=N_CORES),
                        )
                    nc.gpsimd.collective_compute(
                        "AllToAll",
                        mybir.AluOpType.bypass,
                        replica_groups=ALL_CORES,
                        ins=[a2a_in[j].opt()],
                        outs=[a2a_out[j].opt()],
                    )
                    # out-projection of the previous block overlaps this
                    # block's a2a; blocks 0..j-1's a2a overlapped attention.
                    if j > 0:
                        outproj(j - 1)
                outproj(NJ - 1)

    nc.compile()
    return nc


def host_prep(x, wq, wk, wv, wo, S):
    """Build the 8 per-core input maps (numpy, bf16)."""
    import ml_dtypes

    bf = ml_dtypes.bfloat16
    perm = np.concatenate(
        [np.arange(0, ROPE, 2), np.arange(1, ROPE, 2), np.arange(ROPE, HEAD_DIM)]
    )
    wq_p = wq.reshape(N_HEADS, HEAD_DIM, D_MODEL)[:, perm, :]
    wk_p = wk.reshape(N_KV, HEAD_DIM, D_MODEL)[:, perm, :]

    inv = THETA ** (-np.arange(0, ROPE, 2, dtype=np.float64) / ROPE)  # [32]
    t = np.arange(S, dtype=np.float64)
    ang = np.outer(inv, t)  # [32, S]
    cosT, sinT = np.cos(ang), np.sin(ang)
    cos = np.ascontiguousarray(np.concatenate([cosT, cosT], 0)).astype(bf)  # [64,S]
    sin = np.ascontiguousarray(np.concatenate([-sinT, sinT], 0)).astype(bf)  # [64,S]

    # causal masks for diagonal tiles: r = k_chunk - 4*j in [0,4)
    kk = np.arange(128)[:, None]
    qq = np.arange(512)[None, :]
    masks = np.stack(
        [(128 * r + kk <= qq) for r in range(4)]
    ).astype(bf)  # [4,128,512]

    ident = np.eye(128, dtype=np.float32).astype(bf)
    wot = np.ascontiguousarray(wo.T).astype(bf)  # [D in, D out]

    in_maps = []
    for c in range(N_CORES):
        b, g = divmod(c, 4)
        xt = np.ascontiguousarray(x[b, :S].T).astype(bf)
        wqt = np.ascontiguousarray(
            wq_p[HG * g : HG * (g + 1)].reshape(GD, D_MODEL).T
        ).astype(bf)
        wkt = np.ascontiguousarray(wk_p[g].T).astype(bf)
        wvt = np.ascontiguousarray(wv[HEAD_DIM * g : HEAD_DIM * (g + 1)].T).astype(bf)
        in_maps.append(
            {
                "xt": xt,
                "wqt": wqt,
                "wkt": wkt,
                "wvt": wvt,
                "wot": wot,
                "cos": cos,
                "sin": sin,
                "masks": masks,
                "ident": ident,
            }
        )
    return in_maps


def run(x, wq, wk, wv, wo, S=None, trace=False):
    from concourse.bass_utils import run_bass_kernel_spmd

    if S is None:
        S = x.shape[1]
    if S not in _BUILD_CACHE:
        _BUILD_CACHE[S] = build_kernel(S)
    nc = _BUILD_CACHE[S]
    in_maps = host_prep(x, wq, wk, wv, wo, S)
    res = run_bass_kernel_spmd(nc, in_maps, core_ids=list(range(N_CORES)), trace=trace)
    out = np.empty((B, S, D_MODEL), np.float32)
    nj = S // 512
    rb = 512 // N_CORES
    for d in range(N_CORES):
        o = res.results[d]["out"]  # [nj*128, D]
        for j in range(nj):
            for b in range(B):
                out[b, 512 * j + rb * d : 512 * j + rb * (d + 1), :] = o[
                    128 * j + rb * b : 128 * j + rb * (b + 1)
                ]
    return out, res


def kernel(x, wq, wk, wv, wo):
    x = np.asarray(x, np.float32)
    wq = np.asarray(wq, np.float32)
    wk = np.asarray(wk, np.float32)
    wv = np.asarray(wv, np.float32)
    wo = np.asarray(wo, np.float32)
    out, _ = run(x, wq, wk, wv, wo)
    return out


# revision 14
# speedup vs baseline: 1.4085x; 1.0024x over previous
"""GroupedQueryAttention Trainium2 kernel (8 NeuronCores, SPMD).

Sharding: core c -> (batch b = c // 4, kv-group g = c % 4).
Each core computes q/k/v projections for its 4 query heads + 1 kv head,
partial-RoPE, and causal attention for its heads over the full sequence.
Attention outputs are resharded with 8-way AllToAlls (one per seq block,
fired as soon as that block's heads finish, overlapped with later
blocks): for each block, core d ends up owning width/8 rows of BOTH
batches with all 16 heads' features, then runs the full out-projection
for those rows (full wo on every core, no reduction needed).

Blocks: three 512-row blocks + two 256-row half-blocks at the end, so
the final (serial) a2a + out-projection tail is halved. A tiny dummy
AllToAll at kernel start warms the collective path (first-call firmware
overhead + rank sync) off the critical path.

All device matmuls run in bf16 (fp32 PSUM accumulation). The host
pre-chunks operands so every load is a single fully-contiguous DMA and
the contraction dim lands on SBUF partitions (V is transposed on the PE
array):
  xt   [NJ, 128, 16, 512]  x[b].T per (block, partition, chunk)
  wqt  [128, 16, 512]      perm(wq)[group].T chunked (RoPE-deinterleaved)
  wkt  [128, 16, 128]      perm(wk)[group].T chunked
  wvt  [128, 16, 128]      wv[group].T chunked
  wot  [128, 16, 2048]     full wo.T chunked (same on every core)
The RoPE deinterleave permutation reorders each head's first 64 dims to
[evens, odds]; since q and k use the same permutation, q.k dot products
are unchanged and it never needs undoing.
"""

import math
import sys

sys.path.insert(0, "/opt/trn_rl_repo")

import numpy as np  # noqa: E402

D_MODEL = 2048
N_HEADS = 16
N_KV = 4
HEAD_DIM = 128
ROPE = 64
THETA = 10000.0
B = 2
HG = N_HEADS // N_KV  # 4 query heads per kv group
GD = HG * HEAD_DIM  # 512 o-features per group
N_CORES = 8
ALL_CORES = [list(range(N_CORES))]

_BUILD_CACHE: dict = {}


def seq_blocks(S):
    """(row0, width) attention/a2a blocks: 512-wide, last split in two."""
    blocks = [(r, 512) for r in range(0, S - 512, 512)]
    blocks += [(S - 512, 256), (S - 256, 256)]
    return blocks


def build_kernel(S: int):
    """Build the per-core Bass program for sequence length S (multiple of 512)."""
    import concourse.bass as bass
    import concourse.mybir as mybir
    import concourse.tile as tile
    from concourse import bacc

    assert S % 512 == 0
    P = 128
    QT = 512  # max q tile (free dim of scoresT)
    NJ = S // QT  # 512-row seq blocks
    NO = D_MODEL // P  # contraction chunks for projections (16)
    NS = S // P  # seq chunks of 128
    bf16 = mybir.dt.bfloat16
    f32 = mybir.dt.float32
    scale = 1.0 / math.sqrt(HEAD_DIM)
    blocks = seq_blocks(S)

    nc = bacc.Bacc(None, target_bir_lowering=False, debug=False, num_devices=N_CORES)

    xt_d = nc.declare_dram_parameter("xt", [NJ, P, NO, QT], bf16, isOutput=False)
    wqt_d = nc.declare_dram_parameter("wqt", [P, NO, GD], bf16, isOutput=False)
    wkt_d = nc.declare_dram_parameter("wkt", [P, NO, HEAD_DIM], bf16, isOutput=False)
    wvt_d = nc.declare_dram_parameter("wvt", [P, NO, HEAD_DIM], bf16, isOutput=False)
    wot_d = nc.declare_dram_parameter("wot", [P, NO, D_MODEL], bf16, isOutput=False)
    cos_d = nc.declare_dram_parameter("cos", [ROPE, S], bf16, isOutput=False)
    sin_d = nc.declare_dram_parameter("sin", [ROPE, S], bf16, isOutput=False)
    msk_d = nc.declare_dram_parameter("masks", [4, P, QT], bf16, isOutput=False)
    idn_d = nc.declare_dram_parameter("ident", [P, P], bf16, isOutput=False)
    # per-core output rows; see host gather for the row mapping
    out_d = nc.declare_dram_parameter("out", [S // 4, D_MODEL], f32, isOutput=True)

    with tile.TileContext(nc) as tc:
        with (
            tc.tile_pool(name="persist", bufs=1) as persist,
            tc.tile_pool(name="dram", bufs=1, space="DRAM") as dram,
        ):
            # ---- persistent SBUF state ----
            q_sb = persist.tile([P, HG, S], bf16)  # qT, per-head chunks
            k_sb = persist.tile([P, S], bf16)  # kT
            v_sb = persist.tile([P, NS, HEAD_DIM], bf16)  # v natural
            cos_sb = persist.tile([ROPE, S], bf16)
            sin_sb = persist.tile([ROPE, S], bf16)
            msk_sb = persist.tile([P, 4, QT], bf16)
            ones_sb = persist.tile([P, P], bf16)
            idn_sb = persist.tile([P, P], bf16)
            wot_sb = persist.tile([P, NO, D_MODEL], bf16)  # full wo.T, chunked

            a2a_in = [
                dram.tile([N_CORES * GD, w // N_CORES], bf16, name=f"a2ain{i}")
                for i, (_, w) in enumerate(blocks)
            ]
            a2a_out = [
                dram.tile([N_CORES * GD, w // N_CORES], bf16, name=f"a2aout{i}")
                for i, (_, w) in enumerate(blocks)
            ]
            wrm_in = dram.tile([N_CORES * P, 2], bf16)
            wrm_out = dram.tile([N_CORES * P, 2], bf16)

            nc.vector.memset(ones_sb[:], 1.0)
            # warm the collective path (ncfw first-call + rank sync) early
            nc.sync.dma_start(
                wrm_in.rearrange("(c p) r -> p c r", p=P),
                ones_sb[:, 0 : 2 * N_CORES].rearrange("p (c r) -> p c r", r=2),
            )
            nc.gpsimd.collective_compute(
                "AllToAll",
                mybir.AluOpType.bypass,
                replica_groups=ALL_CORES,
                ins=[wrm_in.opt()],
                outs=[wrm_out.opt()],
            )

            # ---- phase 1: projections (+RoPE) ----
            with (
                tc.tile_pool(name="proj_sb", bufs=1) as proj_sb,
                tc.tile_pool(name="proj_ps", bufs=4, space="PSUM") as proj_ps,
                tc.tile_pool(name="vt_ps", bufs=2, space="PSUM") as vt_ps,
                tc.tile_pool(name="rope_tmp", bufs=2) as rtmp,
            ):
                xt_sb = proj_sb.tile([P, NJ, NO, QT], bf16)
                wqt_sb = proj_sb.tile([P, NO, GD], bf16)
                wkt_sb = proj_sb.tile([P, NO, HEAD_DIM], bf16)
                wvt_sb = proj_sb.tile([P, NO, HEAD_DIM], bf16)
                vt_sb = proj_sb.tile([P, S], bf16)

                # loads in dependency order; all fully-contiguous runs
                nc.sync.dma_start(wkt_sb[:], wkt_d[:])
                nc.sync.dma_start(xt_sb[:, 0], xt_d[0])
                nc.sync.dma_start(cos_sb[:], cos_d[:])
                nc.sync.dma_start(sin_sb[:], sin_d[:])
                nc.sync.dma_start(msk_sb[:], msk_d.rearrange("r p q -> p r q"))
                nc.sync.dma_start(idn_sb[:], idn_d[:])
                nc.sync.dma_start(wvt_sb[:], wvt_d[:])
                for j in range(1, NJ):
                    nc.sync.dma_start(xt_sb[:, j], xt_d[j])
                nc.sync.dma_start(wqt_sb[:], wqt_d[:])
                nc.sync.dma_start(wot_sb[:], wot_d[:])

                def rope(dst, jsl):
                    # rotate-half form on deinterleaved rows:
                    #   rows 0:32 = a (even dims), 32:64 = b (odd dims)
                    #   new[0:64] = old[0:64]*cos64 + swap(old[0:64])*sin64
                    # with cos64 = [cosT; cosT], sin64 = [-sinT; sinT].
                    xs = rtmp.tile([64, QT], bf16, tag="xs")
                    nc.vector.tensor_copy(xs[0:32, :], dst[32:64, jsl])
                    nc.vector.tensor_copy(xs[32:64, :], dst[0:32, jsl])
                    t = rtmp.tile([64, QT], bf16, tag="t")
                    u = rtmp.tile([64, QT], bf16, tag="u")
                    nc.vector.tensor_mul(t[:], xs[:], sin_sb[:, jsl])
                    nc.vector.tensor_mul(u[:], dst[0:64, jsl], cos_sb[:, jsl])
                    nc.vector.tensor_add(dst[0:64, jsl], u[:], t[:])

                # kT = wkt.T @ xt  -> [128 dk, S]
                for j in range(NJ):
                    jsl = bass.ts(j, QT)
                    ps = proj_ps.tile([P, QT], f32, tag="ps")
                    for o in range(NO):
                        nc.tensor.matmul(
                            ps[:],
                            wkt_sb[:, o, :],
                            xt_sb[:, j, o, :],
                            start=(o == 0),
                            stop=(o == NO - 1),
                        )
                    nc.vector.tensor_copy(k_sb[:, jsl], ps[:])
                    rope(k_sb, jsl)

                # vT = wvt.T @ xt -> [128 dv, S]; then PE-transpose to v natural
                for j in range(NJ):
                    jsl = bass.ts(j, QT)
                    ps = proj_ps.tile([P, QT], f32, tag="ps")
                    for o in range(NO):
                        nc.tensor.matmul(
                            ps[:],
                            wvt_sb[:, o, :],
                            xt_sb[:, j, o, :],
                            start=(o == 0),
                            stop=(o == NO - 1),
                        )
                    nc.vector.tensor_copy(vt_sb[:, jsl], ps[:])
                    for cc in range(QT // P):
                        c = j * (QT // P) + cc
                        tp = vt_ps.tile([P, P], bf16, tag="vtp")
                        nc.tensor.transpose(tp[:], vt_sb[:, bass.ts(c, P)], idn_sb[:])
                        nc.scalar.copy(v_sb[:, c, :], tp[:])

                # qT per head -> [128 dq, S] x4
                for h in range(HG):
                    for j in range(NJ):
                        jsl = bass.ts(j, QT)
                        ps = proj_ps.tile([P, QT], f32, tag="ps")
                        for o in range(NO):
                            nc.tensor.matmul(
                                ps[:],
                                wqt_sb[:, o, bass.ts(h, P)],
                                xt_sb[:, j, o, :],
                                start=(o == 0),
                                stop=(o == NO - 1),
                            )
                        nc.vector.tensor_copy(q_sb[:, h, jsl], ps[:])
                        rope(q_sb[:, h, :], jsl)

            # ---- phase 2: attention (block-outer) + per-block AllToAll +
            #      interleaved out-projection ----
            with (
                tc.tile_pool(name="attn_sb", bufs=3) as attn_sb,
                tc.tile_pool(name="osb", bufs=2) as osb_pool,
                tc.tile_pool(name="og", bufs=2) as og_pool,
                tc.tile_pool(name="fin", bufs=2) as fin,
                tc.tile_pool(name="ps_sc", bufs=2, space="PSUM") as ps_sc,
                tc.tile_pool(name="ps_acc", bufs=1, space="PSUM") as ps_acc,
                tc.tile_pool(name="ps_sum", bufs=1, space="PSUM") as ps_sum,
                tc.tile_pool(name="ps_out", bufs=2, space="PSUM") as ps_out,
            ):

                def outproj(jblk):
                    # 128 output rows of 512-row block jblk (64 per batch),
                    # full 2048-feature contraction, results DMA'd per n.
                    # og[p, 4g+fi, col] with col = 64*b + r-within-batch.
                    og_sb = og_pool.tile([P, NO, P], bf16, tag="og")
                    ncol = 0
                    for i, (row0, w) in enumerate(blocks):
                        if not (512 * jblk <= row0 < 512 * (jblk + 1)):
                            continue
                        rb = w // N_CORES
                        src = a2a_out[i].rearrange(
                            "(b gfi p) r -> b p gfi r", b=B, p=P
                        )
                        for b in range(B):
                            nc.sync.dma_start(
                                og_sb[:, :, 64 * b + ncol : 64 * b + ncol + rb],
                                src[b],
                            )
                        ncol += rb
                    ot = fin.tile([P, D_MODEL], f32, tag="ot")
                    for n in range(D_MODEL // QT):
                        ps = ps_out.tile([P, QT], f32, tag="pout")
                        for c in range(NO):
                            nc.tensor.matmul(
                                ps[:],
                                og_sb[:, c, :],
                                wot_sb[:, c, bass.ts(n, QT)],
                                start=(c == 0),
                                stop=(c == NO - 1),
                            )
                        nsl = bass.ts(n, QT)
                        nc.scalar.copy(ot[:, nsl], ps[:])
                        nc.sync.dma_start(out_d[bass.ts(jblk, P), nsl], ot[:, nsl])

                done_oproj = 0
                for i, (row0, w) in enumerate(blocks):
                    nk = (row0 + w) // P  # causal: k chunks 0..nk-1
                    w2 = 2 * w
                    o_sb = osb_pool.tile([P, HG, QT], bf16, tag="osb")
                    for h in range(HG):
                        po = ps_acc.tile([P, QT], f32, tag="po")
                        acc = attn_sb.tile([P, QT], bf16, tag="acc")
                        for c2 in range(nk // 2):
                            sc = ps_sc.tile([P, 2 * QT], f32, tag="sc")
                            pt = attn_sb.tile([P, 2 * QT], bf16, tag="pt")
                            for t in (0, 1):
                                c = 2 * c2 + t
                                nc.tensor.matmul(
                                    sc[:, t * w : (t + 1) * w],
                                    k_sb[:, bass.ts(c, P)],
                                    q_sb[:, h, row0 : row0 + w],
                                    start=True,
                                    stop=True,
                                )
                            nc.scalar.activation(
                                pt[:, 0:w2],
                                sc[:, 0:w2],
                                mybir.ActivationFunctionType.Exp,
                                scale=scale,
                            )
                            for t in (0, 1):
                                c = 2 * c2 + t
                                r = c - row0 // P
                                if 0 <= r < w // P:
                                    nc.vector.tensor_mul(
                                        pt[:, t * w : (t + 1) * w],
                                        pt[:, t * w : (t + 1) * w],
                                        msk_sb[:, r, 0:w],
                                    )
                            # softmax-denominator partials accumulate on DVE
                            for t in (0, 1):
                                c = 2 * c2 + t
                                if c == 0:
                                    nc.vector.tensor_copy(
                                        acc[:, 0:w], pt[:, 0:w]
                                    )
                                else:
                                    nc.vector.tensor_add(
                                        acc[:, 0:w],
                                        acc[:, 0:w],
                                        pt[:, t * w : (t + 1) * w],
                                    )
                            for t in (0, 1):
                                c = 2 * c2 + t
                                nc.tensor.matmul(
                                    po[:, 0:w],
                                    v_sb[:, c, :],
                                    pt[:, t * w : (t + 1) * w],
                                    start=(c == 0),
                                    stop=(c == nk - 1),
                                )
                        # partition-reduce acc + broadcast in one matmul
                        rs = ps_sum.tile([P, QT], f32, tag="rs")
                        nc.tensor.matmul(
                            rs[:, 0:w], ones_sb[:], acc[:, 0:w], start=True, stop=True
                        )
                        rcp = attn_sb.tile([P, QT], f32, tag="rcp")
                        nc.vector.reciprocal_approx_fast(rcp[:, 0:w], rs[:, 0:w])
                        nc.vector.tensor_mul(o_sb[:, h, 0:w], po[:, 0:w], rcp[:, 0:w])

                    # ship block i: a2a_in[i][(d*4+h)*128+p, r] = o_sb[p,h,rb*d+r]
                    in_v = a2a_in[i].rearrange("(d h p) r -> p h d r", h=HG, p=P)
                    for h in range(HG):
                        nc.sync.dma_start(
                            in_v[:, h],
                            o_sb[:, h, 0:w].rearrange("p (d r) -> p d r", d=N_CORES),
                        )
                    nc.gpsimd.collective_compute(
                        "AllToAll",
                        mybir.AluOpType.bypass,
                        replica_groups=ALL_CORES,
                        ins=[a2a_in[i].opt()],
                        outs=[a2a_out[i].opt()],
                    )
                    # out-projection of a finished 512-row block overlaps this
                    # block's a2a
                    ready = row0 // 512  # 512-blocks fully shipped before i
                    if done_oproj < ready:
                        outproj(done_oproj)
                        done_oproj += 1
                while done_oproj < NJ:
                    outproj(done_oproj)
                    done_oproj += 1

    nc.compile()
    return nc


def host_prep(x, wq, wk, wv, wo, S):
    """Build the 8 per-core input maps (numpy, bf16)."""
    import ml_dtypes

    bf = ml_dtypes.bfloat16
    NJ = S // 512
    NO = D_MODEL // 128
    perm = np.concatenate(
        [np.arange(0, ROPE, 2), np.arange(1, ROPE, 2), np.arange(ROPE, HEAD_DIM)]
    )
    wq_p = wq.reshape(N_HEADS, HEAD_DIM, D_MODEL)[:, perm, :]
    wk_p = wk.reshape(N_KV, HEAD_DIM, D_MODEL)[:, perm, :]

    inv = THETA ** (-np.arange(0, ROPE, 2, dtype=np.float64) / ROPE)  # [32]
    t = np.arange(S, dtype=np.float64)
    ang = np.outer(inv, t)  # [32, S]
    cosT, sinT = np.cos(ang), np.sin(ang)
    cos = np.ascontiguousarray(np.concatenate([cosT, cosT], 0)).astype(bf)  # [64,S]
    sin = np.ascontiguousarray(np.concatenate([-sinT, sinT], 0)).astype(bf)  # [64,S]

    # causal masks for diagonal tiles: r = k_chunk - row0/128 in [0,4)
    kk = np.arange(128)[:, None]
    qq = np.arange(512)[None, :]
    masks = np.stack(
        [(128 * r + kk <= qq) for r in range(4)]
    ).astype(bf)  # [4,128,512]

    ident = np.eye(128, dtype=np.float32).astype(bf)

    def chunked(wT):  # [D, M] -> [128, NO, M] contiguous
        return np.ascontiguousarray(
            wT.reshape(NO, 128, wT.shape[1]).transpose(1, 0, 2)
        ).astype(bf)

    wot = chunked(wo.T.astype(np.float32))

    in_maps = []
    for c in range(N_CORES):
        b, g = divmod(c, 4)
        # xt[j, p, o, s] = x[b, 512j+s, 128o+p]
        xt = np.ascontiguousarray(
            x[b, :S].reshape(NJ, 512, NO, 128).transpose(0, 3, 2, 1)
        ).astype(bf)
        wqt = chunked(wq_p[HG * g : HG * (g + 1)].reshape(GD, D_MODEL).T)
        wkt = chunked(wk_p[g].T)
        wvt = chunked(wv[HEAD_DIM * g : HEAD_DIM * (g + 1)].T)
        in_maps.append(
            {
                "xt": xt,
                "wqt": wqt,
                "wkt": wkt,
                "wvt": wvt,
                "wot": wot,
                "cos": cos,
                "sin": sin,
                "masks": masks,
                "ident": ident,
            }
        )
    return in_maps


def core_rows(S, d):
    """Device out_d row index -> (batch, seq) for core d, in device order."""
    rows = []
    for jblk in range(S // 512):
        row0 = 512 * jblk
        for b in range(B):
            if jblk < S // 512 - 1:
                rows += [(b, row0 + 64 * d + r) for r in range(64)]
            else:
                rows += [
                    (b, row0 + 256 * half + 32 * d + r)
                    for half in range(2)
                    for r in range(32)
                ]
    return rows


def run(x, wq, wk, wv, wo, S=None, trace=False):
    from concourse.bass_utils import run_bass_kernel_spmd

    if S is None:
        S = x.shape[1]
    if S not in _BUILD_CACHE:
        _BUILD_CACHE[S] = build_kernel(S)
    nc = _BUILD_CACHE[S]
    in_maps = host_prep(x, wq, wk, wv, wo, S)
    res = run_bass_kernel_spmd(nc, in_maps, core_ids=list(range(N_CORES)), trace=trace)
    out = np.empty((B, S, D_MODEL), np.float32)
    for d in range(N_CORES):
        o = res.results[d]["out"]  # [S//4, D]
        rows = core_rows(S, d)
        bs = np.array([r[0] for r in rows])
        ss = np.array([r[1] for r in rows])
        out[bs, ss, :] = o
    return out, res


def kernel(x, wq, wk, wv, wo):
    x = np.asarray(x, np.float32)
    wq = np.asarray(wq, np.float32)
    wk = np.asarray(wk, np.float32)
    wv = np.asarray(wv, np.float32)
    wo = np.asarray(wo, np.float32)
    out, _ = run(x, wq, wk, wv, wo)
    return out


# revision 16
# speedup vs baseline: 1.4366x; 1.0200x over previous
"""GroupedQueryAttention Trainium2 kernel (8 NeuronCores, SPMD).

Sharding: core c -> (batch b = c // 4, kv-group g = c % 4).
Each core computes q/k/v projections for its 4 query heads + 1 kv head,
partial-RoPE, and causal attention for its heads over the full sequence.
Attention outputs are resharded with 8-way AllToAlls (one per seq block,
fired as soon as that block's heads finish, overlapped with later
blocks): for each block, core d ends up owning width/8 rows of BOTH
batches with all 16 heads' features, then runs the full out-projection
for those rows (full wo on every core, no reduction needed).

Blocks: three 512-row blocks + two 256-row half-blocks at the end, so
the final (serial) a2a + out-projection tail is halved. A tiny dummy
AllToAll at kernel start warms the collective path (first-call firmware
overhead + rank sync) off the critical path.

All device matmuls run in bf16 (fp32 PSUM accumulation). The host
pre-chunks operands so every load is a single fully-contiguous DMA and
the contraction dim lands on SBUF partitions (V is transposed on the PE
array):
  xt   [NJ, 128, 16, 512]  x[b].T per (block, partition, chunk)
  wqt  [128, 16, 512]      perm(wq)[group].T chunked (RoPE-deinterleaved)
  wkt  [128, 16, 128]      perm(wk)[group].T chunked
  wvt  [128, 16, 128]      wv[group].T chunked
  wot  [128, 16, 2048]     full wo.T chunked (same on every core)
The RoPE deinterleave permutation reorders each head's first 64 dims to
[evens, odds]; since q and k use the same permutation, q.k dot products
are unchanged and it never needs undoing.
"""

import math
import sys

sys.path.insert(0, "/opt/trn_rl_repo")

import numpy as np  # noqa: E402

D_MODEL = 2048
N_HEADS = 16
N_KV = 4
HEAD_DIM = 128
ROPE = 64
THETA = 10000.0
B = 2
HG = N_HEADS // N_KV  # 4 query heads per kv group
GD = HG * HEAD_DIM  # 512 o-features per group
N_CORES = 8
ALL_CORES = [list(range(N_CORES))]

_BUILD_CACHE: dict = {}


def seq_blocks(S):
    """(row0, width) attention/a2a blocks: 512-wide, last split in two."""
    blocks = [(r, 512) for r in range(0, S - 512, 512)]
    blocks += [(S - 512, 256), (S - 256, 256)]
    return blocks


def build_kernel(S: int):
    """Build the per-core Bass program for sequence length S (multiple of 512)."""
    import concourse.bass as bass
    import concourse.mybir as mybir
    import concourse.tile as tile
    from concourse import bacc

    assert S % 512 == 0
    P = 128
    QT = 512  # max q tile (free dim of scoresT)
    NJ = S // QT  # 512-row seq blocks
    NO = D_MODEL // P  # contraction chunks for projections (16)
    NS = S // P  # seq chunks of 128
    bf16 = mybir.dt.bfloat16
    f32 = mybir.dt.float32
    scale = 1.0 / math.sqrt(HEAD_DIM)
    blocks = seq_blocks(S)

    nc = bacc.Bacc(None, target_bir_lowering=False, debug=False, num_devices=N_CORES)

    xt_d = nc.declare_dram_parameter("xt", [NJ, P, NO, QT], bf16, isOutput=False)
    wqt_d = nc.declare_dram_parameter("wqt", [P, NO, GD], bf16, isOutput=False)
    wkt_d = nc.declare_dram_parameter("wkt", [P, NO, HEAD_DIM], bf16, isOutput=False)
    wvt_d = nc.declare_dram_parameter("wvt", [P, NO, HEAD_DIM], bf16, isOutput=False)
    wot_d = nc.declare_dram_parameter("wot", [P, NO, D_MODEL], bf16, isOutput=False)
    cos_d = nc.declare_dram_parameter("cos", [ROPE, S], bf16, isOutput=False)
    sin_d = nc.declare_dram_parameter("sin", [ROPE, S], bf16, isOutput=False)
    msk_d = nc.declare_dram_parameter("masks", [4, P, QT], bf16, isOutput=False)
    idn_d = nc.declare_dram_parameter("ident", [P, P], bf16, isOutput=False)
    # per-core output rows; see host gather for the row mapping
    out_d = nc.declare_dram_parameter("out", [S // 4, D_MODEL], f32, isOutput=True)

    with tile.TileContext(nc) as tc:
        with (
            tc.tile_pool(name="persist", bufs=1) as persist,
            tc.tile_pool(name="dram", bufs=1, space="DRAM") as dram,
        ):
            # ---- persistent SBUF state ----
            q_sb = persist.tile([P, HG, S], bf16)  # qT, per-head chunks
            k_sb = persist.tile([P, S], bf16)  # kT
            v_sb = persist.tile([P, NS, HEAD_DIM], bf16)  # v natural
            cos_sb = persist.tile([ROPE, S], bf16)
            sin_sb = persist.tile([ROPE, S], bf16)
            msk_sb = persist.tile([P, 4, QT], bf16)
            ones_sb = persist.tile([P, P], bf16)
            idn_sb = persist.tile([P, P], bf16)
            wot_sb = persist.tile([P, NO, D_MODEL], bf16)  # full wo.T, chunked

            a2a_in = [
                dram.tile([N_CORES * GD, w // N_CORES], bf16, name=f"a2ain{i}")
                for i, (_, w) in enumerate(blocks)
            ]
            a2a_out = [
                dram.tile([N_CORES * GD, w // N_CORES], bf16, name=f"a2aout{i}")
                for i, (_, w) in enumerate(blocks)
            ]
            wrm_in = dram.tile([P, 16], bf16)
            wrm_out = dram.tile([P, 16], bf16)

            nc.vector.memset(ones_sb[:], 1.0)

            # ---- phase 1: projections (+RoPE) ----
            with (
                tc.tile_pool(name="proj_sb", bufs=1) as proj_sb,
                tc.tile_pool(name="proj_ps", bufs=4, space="PSUM") as proj_ps,
                tc.tile_pool(name="vt_ps", bufs=2, space="PSUM") as vt_ps,
                tc.tile_pool(name="rope_tmp", bufs=2) as rtmp,
            ):
                xt_sb = proj_sb.tile([P, NJ, NO, QT], bf16)
                wqt_sb = proj_sb.tile([P, NO, GD], bf16)
                wkt_sb = proj_sb.tile([P, NO, HEAD_DIM], bf16)
                wvt_sb = proj_sb.tile([P, NO, HEAD_DIM], bf16)
                vt_sb = proj_sb.tile([P, S], bf16)

                # loads in dependency order; all fully-contiguous runs.
                # Tiny/strided DMAs (warmup seed, masks, ident) come AFTER the
                # first xt blocks — their small packets clog the single DMA
                # queue's FIFO ahead of the data the first matmuls need.
                nc.sync.dma_start(wkt_sb[:], wkt_d[:])
                nc.sync.dma_start(xt_sb[:, 0], xt_d[0])
                nc.sync.dma_start(cos_sb[:], cos_d[:])
                nc.sync.dma_start(sin_sb[:], sin_d[:])
                # warm the collective path (ncfw first-call + rank sync) early
                nc.sync.dma_start(wrm_in[:], ones_sb[:, 0:16])
                nc.gpsimd.collective_compute(
                    "AllToAll",
                    mybir.AluOpType.bypass,
                    replica_groups=ALL_CORES,
                    ins=[wrm_in.opt()],
                    outs=[wrm_out.opt()],
                )
                nc.sync.dma_start(wvt_sb[:], wvt_d[:])
                nc.sync.dma_start(xt_sb[:, 1], xt_d[1])
                nc.sync.dma_start(xt_sb[:, 2], xt_d[2])
                nc.sync.dma_start(msk_sb[:], msk_d.rearrange("r p q -> p r q"))
                nc.sync.dma_start(idn_sb[:], idn_d[:])
                for j in range(3, NJ):
                    nc.sync.dma_start(xt_sb[:, j], xt_d[j])
                nc.sync.dma_start(wqt_sb[:], wqt_d[:])
                nc.sync.dma_start(wot_sb[:], wot_d[:])

                def rope(dst, jsl):
                    # rotate-half form on deinterleaved rows:
                    #   rows 0:32 = a (even dims), 32:64 = b (odd dims)
                    #   new[0:64] = old[0:64]*cos64 + swap(old[0:64])*sin64
                    # with cos64 = [cosT; cosT], sin64 = [-sinT; sinT].
                    xs = rtmp.tile([64, QT], bf16, tag="xs")
                    nc.vector.tensor_copy(xs[0:32, :], dst[32:64, jsl])
                    nc.vector.tensor_copy(xs[32:64, :], dst[0:32, jsl])
                    t = rtmp.tile([64, QT], bf16, tag="t")
                    u = rtmp.tile([64, QT], bf16, tag="u")
                    nc.vector.tensor_mul(t[:], xs[:], sin_sb[:, jsl])
                    nc.vector.tensor_mul(u[:], dst[0:64, jsl], cos_sb[:, jsl])
                    nc.vector.tensor_add(dst[0:64, jsl], u[:], t[:])

                # kT = wkt.T @ xt  -> [128 dk, S]
                for j in range(NJ):
                    jsl = bass.ts(j, QT)
                    ps = proj_ps.tile([P, QT], f32, tag="ps")
                    for o in range(NO):
                        nc.tensor.matmul(
                            ps[:],
                            wkt_sb[:, o, :],
                            xt_sb[:, j, o, :],
                            start=(o == 0),
                            stop=(o == NO - 1),
                        )
                    nc.vector.tensor_copy(k_sb[:, jsl], ps[:])
                    rope(k_sb, jsl)

                # vT = wvt.T @ xt -> [128 dv, S]; then PE-transpose to v natural
                for j in range(NJ):
                    jsl = bass.ts(j, QT)
                    ps = proj_ps.tile([P, QT], f32, tag="ps")
                    for o in range(NO):
                        nc.tensor.matmul(
                            ps[:],
                            wvt_sb[:, o, :],
                            xt_sb[:, j, o, :],
                            start=(o == 0),
                            stop=(o == NO - 1),
                        )
                    nc.vector.tensor_copy(vt_sb[:, jsl], ps[:])
                    for cc in range(QT // P):
                        c = j * (QT // P) + cc
                        tp = vt_ps.tile([P, P], bf16, tag="vtp")
                        nc.tensor.transpose(tp[:], vt_sb[:, bass.ts(c, P)], idn_sb[:])
                        nc.scalar.copy(v_sb[:, c, :], tp[:])

                # qT per head -> [128 dq, S] x4
                for h in range(HG):
                    for j in range(NJ):
                        jsl = bass.ts(j, QT)
                        ps = proj_ps.tile([P, QT], f32, tag="ps")
                        for o in range(NO):
                            nc.tensor.matmul(
                                ps[:],
                                wqt_sb[:, o, bass.ts(h, P)],
                                xt_sb[:, j, o, :],
                                start=(o == 0),
                                stop=(o == NO - 1),
                            )
                        nc.vector.tensor_copy(q_sb[:, h, jsl], ps[:])
                        rope(q_sb[:, h, :], jsl)

            # ---- phase 2: attention (block-outer) + per-block AllToAll +
            #      interleaved out-projection ----
            with (
                tc.tile_pool(name="attn_sb", bufs=3) as attn_sb,
                tc.tile_pool(name="osb", bufs=2) as osb_pool,
                tc.tile_pool(name="og", bufs=2) as og_pool,
                tc.tile_pool(name="fin", bufs=2) as fin,
                tc.tile_pool(name="ps_sc", bufs=2, space="PSUM") as ps_sc,
                tc.tile_pool(name="ps_acc", bufs=1, space="PSUM") as ps_acc,
                tc.tile_pool(name="ps_sum", bufs=1, space="PSUM") as ps_sum,
                tc.tile_pool(name="ps_out", bufs=2, space="PSUM") as ps_out,
            ):

                def outproj(jblk):
                    # 128 output rows of 512-row block jblk (64 per batch),
                    # full 2048-feature contraction, results DMA'd per n.
                    # og[p, 4g+fi, col] with col = 64*b + r-within-batch.
                    og_sb = og_pool.tile([P, NO, P], bf16, tag="og")
                    ncol = 0
                    for i, (row0, w) in enumerate(blocks):
                        if not (512 * jblk <= row0 < 512 * (jblk + 1)):
                            continue
                        rb = w // N_CORES
                        src = a2a_out[i].rearrange(
                            "(b gfi p) r -> b p gfi r", b=B, p=P
                        )
                        for b in range(B):
                            nc.sync.dma_start(
                                og_sb[:, :, 64 * b + ncol : 64 * b + ncol + rb],
                                src[b],
                            )
                        ncol += rb
                    ot = fin.tile([P, D_MODEL], f32, tag="ot")
                    for n in range(D_MODEL // QT):
                        ps = ps_out.tile([P, QT], f32, tag="pout")
                        for c in range(NO):
                            nc.tensor.matmul(
                                ps[:],
                                og_sb[:, c, :],
                                wot_sb[:, c, bass.ts(n, QT)],
                                start=(c == 0),
                                stop=(c == NO - 1),
                            )
                        nsl = bass.ts(n, QT)
                        nc.scalar.copy(ot[:, nsl], ps[:])
                        nc.sync.dma_start(out_d[bass.ts(jblk, P), nsl], ot[:, nsl])

                done_oproj = 0
                for i, (row0, w) in enumerate(blocks):
                    nk = (row0 + w) // P  # causal: k chunks 0..nk-1
                    w2 = 2 * w
                    o_sb = osb_pool.tile([P, HG, QT], bf16, tag="osb")
                    for h in range(HG):
                        po = ps_acc.tile([P, QT], f32, tag="po")
                        acc = attn_sb.tile([P, QT], bf16, tag="acc")
                        for c2 in range(nk // 2):
                            sc = ps_sc.tile([P, 2 * QT], f32, tag="sc")
                            pt = attn_sb.tile([P, 2 * QT], bf16, tag="pt")
                            for t in (0, 1):
                                c = 2 * c2 + t
                                nc.tensor.matmul(
                                    sc[:, t * w : (t + 1) * w],
                                    k_sb[:, bass.ts(c, P)],
                                    q_sb[:, h, row0 : row0 + w],
                                    start=True,
                                    stop=True,
                                )
                            nc.scalar.activation(
                                pt[:, 0:w2],
                                sc[:, 0:w2],
                                mybir.ActivationFunctionType.Exp,
                                scale=scale,
                            )
                            for t in (0, 1):
                                c = 2 * c2 + t
                                r = c - row0 // P
                                if 0 <= r < w // P:
                                    nc.vector.tensor_mul(
                                        pt[:, t * w : (t + 1) * w],
                                        pt[:, t * w : (t + 1) * w],
                                        msk_sb[:, r, 0:w],
                                    )
                            # softmax-denominator partials accumulate on DVE
                            for t in (0, 1):
                                c = 2 * c2 + t
                                if c == 0:
                                    nc.vector.tensor_copy(
                                        acc[:, 0:w], pt[:, 0:w]
                                    )
                                else:
                                    nc.vector.tensor_add(
                                        acc[:, 0:w],
                                        acc[:, 0:w],
                                        pt[:, t * w : (t + 1) * w],
                                    )
                            for t in (0, 1):
                                c = 2 * c2 + t
                                nc.tensor.matmul(
                                    po[:, 0:w],
                                    v_sb[:, c, :],
                                    pt[:, t * w : (t + 1) * w],
                                    start=(c == 0),
                                    stop=(c == nk - 1),
                                )
                        # partition-reduce acc + broadcast in one matmul
                        rs = ps_sum.tile([P, QT], f32, tag="rs")
                        nc.tensor.matmul(
                            rs[:, 0:w], ones_sb[:], acc[:, 0:w], start=True, stop=True
                        )
                        rcp = attn_sb.tile([P, QT], f32, tag="rcp")
                        nc.vector.reciprocal_approx_fast(rcp[:, 0:w], rs[:, 0:w])
                        nc.vector.tensor_mul(o_sb[:, h, 0:w], po[:, 0:w], rcp[:, 0:w])

                    # ship block i: a2a_in[i][(d*4+h)*128+p, r] = o_sb[p,h,rb*d+r]
                    in_v = a2a_in[i].rearrange("(d h p) r -> p h d r", h=HG, p=P)
                    for h in range(HG):
                        nc.sync.dma_start(
                            in_v[:, h],
                            o_sb[:, h, 0:w].rearrange("p (d r) -> p d r", d=N_CORES),
                        )
                    nc.gpsimd.collective_compute(
                        "AllToAll",
                        mybir.AluOpType.bypass,
                        replica_groups=ALL_CORES,
                        ins=[a2a_in[i].opt()],
                        outs=[a2a_out[i].opt()],
                    )
                    # out-projection of a finished 512-row block overlaps this
                    # block's a2a
                    ready = row0 // 512  # 512-blocks fully shipped before i
                    if done_oproj < ready:
                        outproj(done_oproj)
                        done_oproj += 1
                while done_oproj < NJ:
                    outproj(done_oproj)
                    done_oproj += 1

    nc.compile()
    return nc


def host_prep(x, wq, wk, wv, wo, S):
    """Build the 8 per-core input maps (numpy, bf16)."""
    import ml_dtypes

    bf = ml_dtypes.bfloat16
    NJ = S // 512
    NO = D_MODEL // 128
    perm = np.concatenate(
        [np.arange(0, ROPE, 2), np.arange(1, ROPE, 2), np.arange(ROPE, HEAD_DIM)]
    )
    wq_p = wq.reshape(N_HEADS, HEAD_DIM, D_MODEL)[:, perm, :]
    wk_p = wk.reshape(N_KV, HEAD_DIM, D_MODEL)[:, perm, :]

    inv = THETA ** (-np.arange(0, ROPE, 2, dtype=np.float64) / ROPE)  # [32]
    t = np.arange(S, dtype=np.float64)
    ang = np.outer(inv, t)  # [32, S]
    cosT, sinT = np.cos(ang), np.sin(ang)
    cos = np.ascontiguousarray(np.concatenate([cosT, cosT], 0)).astype(bf)  # [64,S]
    sin = np.ascontiguousarray(np.concatenate([-sinT, sinT], 0)).astype(bf)  # [64,S]

    # causal masks for diagonal tiles: r = k_chunk - row0/128 in [0,4)
    kk = np.arange(128)[:, None]
    qq = np.arange(512)[None, :]
    masks = np.stack(
        [(128 * r + kk <= qq) for r in range(4)]
    ).astype(bf)  # [4,128,512]

    ident = np.eye(128, dtype=np.float32).astype(bf)

    def chunked(wT):  # [D, M] -> [128, NO, M] contiguous
        return np.ascontiguousarray(
            wT.reshape(NO, 128, wT.shape[1]).transpose(1, 0, 2)
        ).astype(bf)

    wot = chunked(wo.T.astype(np.float32))

    in_maps = []
    for c in range(N_CORES):
        b, g = divmod(c, 4)
        # xt[j, p, o, s] = x[b, 512j+s, 128o+p]
        xt = np.ascontiguousarray(
            x[b, :S].reshape(NJ, 512, NO, 128).transpose(0, 3, 2, 1)
        ).astype(bf)
        wqt = chunked(wq_p[HG * g : HG * (g + 1)].reshape(GD, D_MODEL).T)
        wkt = chunked(wk_p[g].T)
        wvt = chunked(wv[HEAD_DIM * g : HEAD_DIM * (g + 1)].T)
        in_maps.append(
            {
                "xt": xt,
                "wqt": wqt,
                "wkt": wkt,
                "wvt": wvt,
                "wot": wot,
                "cos": cos,
                "sin": sin,
                "masks": masks,
                "ident": ident,
            }
        )
    return in_maps


def core_rows(S, d):
    """Device out_d row index -> (batch, seq) for core d, in device order."""
    rows = []
    for jblk in range(S // 512):
        row0 = 512 * jblk
        for b in range(B):
            if jblk < S // 512 - 1:
                rows += [(b, row0 + 64 * d + r) for r in range(64)]
            else:
                rows += [
                    (b, row0 + 256 * half + 32 * d + r)
                    for half in range(2)
                    for r in range(32)
                ]
    return rows


def run(x, wq, wk, wv, wo, S=None, trace=False):
    from concourse.bass_utils import run_bass_kernel_spmd

    if S is None:
        S = x.shape[1]
    if S not in _BUILD_CACHE:
        _BUILD_CACHE[S] = build_kernel(S)
    nc = _BUILD_CACHE[S]
    in_maps = host_prep(x, wq, wk, wv, wo, S)
    res = run_bass_kernel_spmd(nc, in_maps, core_ids=list(range(N_CORES)), trace=trace)
    out = np.empty((B, S, D_MODEL), np.float32)
    for d in range(N_CORES):
        o = res.results[d]["out"]  # [S//4, D]
        rows = core_rows(S, d)
        bs = np.array([r[0] for r in rows])
        ss = np.array([r[1] for r in rows])
        out[bs, ss, :] = o
    return out, res


def kernel(x, wq, wk, wv, wo):
    x = np.asarray(x, np.float32)
    wq = np.asarray(wq, np.float32)
    wk = np.asarray(wk, np.float32)
    wv = np.asarray(wv, np.float32)
    wo = np.asarray(wo, np.float32)
    out, _ = run(x, wq, wk, wv, wo)
    return out


# revision 30
# speedup vs baseline: 1.4516x; 1.0104x over previous
"""GroupedQueryAttention Trainium2 kernel (8 NeuronCores, SPMD).

Sharding: core c -> (batch b = c // 4, kv-group g = c % 4).
Each core computes q/k/v projections for its 4 query heads + 1 kv head,
partial-RoPE, and causal attention for its heads over the full sequence.
Attention outputs are resharded with 8-way AllToAlls (one per seq block,
fired as soon as that block's heads finish, overlapped with later
blocks): for each block, core d ends up owning width/8 rows of BOTH
batches with all 16 heads' features, then runs the full out-projection
for those rows (full wo on every core, no reduction needed).

Blocks: three 512-row blocks + two 256-row half-blocks at the end, so
the final (serial) a2a + out-projection tail is halved. A tiny dummy
AllToAll at kernel start warms the collective path (first-call firmware
overhead + rank sync) off the critical path.

All device matmuls run in bf16 (fp32 PSUM accumulation). The host
pre-chunks operands so every load is a single fully-contiguous DMA and
the contraction dim lands on SBUF partitions (V is transposed on the PE
array):
  xt   [NJ, 128, 16, 512]  x[b].T per (block, partition, chunk)
  wqt  [128, 16, 512]      perm(wq)[group].T chunked (RoPE-deinterleaved)
  wkt  [128, 16, 128]      perm(wk)[group].T chunked
  wvt  [128, 16, 128]      wv[group].T chunked
  wot  [128, 16, 2048]     full wo.T chunked (same on every core)
The RoPE deinterleave permutation reorders each head's first 64 dims to
[evens, odds]; since q and k use the same permutation, q.k dot products
are unchanged and it never needs undoing.
"""

import math
import sys

sys.path.insert(0, "/opt/trn_rl_repo")

import numpy as np  # noqa: E402

D_MODEL = 2048
N_HEADS = 16
N_KV = 4
HEAD_DIM = 128
ROPE = 64
THETA = 10000.0
B = 2
HG = N_HEADS // N_KV  # 4 query heads per kv group
GD = HG * HEAD_DIM  # 512 o-features per group
N_CORES = 8
ALL_CORES = [list(range(N_CORES))]

_BUILD_CACHE: dict = {}


def seq_blocks(S):
    """(row0, width) attention/a2a blocks: 512-wide, last split in two."""
    blocks = [(r, 512) for r in range(0, S - 512, 512)]
    blocks += [(S - 512, 256), (S - 256, 256)]
    return blocks


def build_kernel(S: int):
    """Build the per-core Bass program for sequence length S (multiple of 512)."""
    import concourse.bass as bass
    import concourse.mybir as mybir
    import concourse.tile as tile
    from concourse import bacc

    assert S % 512 == 0
    P = 128
    QT = 512  # max q tile (free dim of scoresT)
    NJ = S // QT  # 512-row seq blocks
    NO = D_MODEL // P  # contraction chunks for projections (16)
    NS = S // P  # seq chunks of 128
    bf16 = mybir.dt.bfloat16
    f32 = mybir.dt.float32
    f8 = mybir.dt.float8e4
    scale = 1.0 / math.sqrt(HEAD_DIM)
    blocks = seq_blocks(S)

    nc = bacc.Bacc(None, target_bir_lowering=False, debug=False, num_devices=N_CORES)

    xt_d = nc.declare_dram_parameter("xt", [NJ, P, NO, QT], bf16, isOutput=False)
    wqt_d = nc.declare_dram_parameter("wqt", [P, NO, GD], bf16, isOutput=False)
    wkt_d = nc.declare_dram_parameter("wkt", [P, NO, HEAD_DIM], bf16, isOutput=False)
    wvt_d = nc.declare_dram_parameter("wvt", [P, NO, HEAD_DIM], bf16, isOutput=False)
    wot_d = nc.declare_dram_parameter("wot", [P, NO, D_MODEL], bf16, isOutput=False)
    cos_d = nc.declare_dram_parameter("cos", [ROPE, S], bf16, isOutput=False)
    sin_d = nc.declare_dram_parameter("sin", [ROPE, S], bf16, isOutput=False)
    msk_d = nc.declare_dram_parameter("masks", [4, P, QT], bf16, isOutput=False)
    idn_d = nc.declare_dram_parameter("ident", [P, P], bf16, isOutput=False)
    # per-core output rows; see host gather for the row mapping
    out_d = nc.declare_dram_parameter("out", [S // 4, D_MODEL], f32, isOutput=True)

    with tile.TileContext(nc) as tc:
        with (
            tc.tile_pool(name="persist", bufs=1) as persist,
            tc.tile_pool(name="dram", bufs=1, space="DRAM") as dram,
        ):
            # ---- persistent SBUF state ----
            q_sb = persist.tile([P, HG, S], bf16)  # qT, per-head chunks
            k_sb = persist.tile([P, S], bf16)  # kT
            v_sb = persist.tile([P, NS, HEAD_DIM], bf16)  # v natural
            # (fp8 DoubleRow PV was tried and reverted: e4m3's ~3.6% rms
            # quantization hits the softmax-weighted mean unattenuated,
            # measured 3.5e-2 rel err vs the 2e-2 budget)
            cos_sb = persist.tile([ROPE, S], bf16)
            sin_sb = persist.tile([ROPE, S], bf16)
            msk_sb = persist.tile([P, 4, QT], bf16)
            ones_sb = persist.tile([P, P], bf16)
            idn_sb = persist.tile([P, P], bf16)
            wot_sb = persist.tile([P, NO, D_MODEL], bf16)  # full wo.T, chunked

            a2a_in = [
                dram.tile([N_CORES * GD, w // N_CORES], bf16, name=f"a2ain{i}")
                for i, (_, w) in enumerate(blocks)
            ]
            a2a_out = [
                dram.tile([N_CORES * GD, w // N_CORES], bf16, name=f"a2aout{i}")
                for i, (_, w) in enumerate(blocks)
            ]
            wrm_in = dram.tile([P, 16], bf16)
            wrm_out = dram.tile([P, 16], bf16)

            nc.vector.memset(ones_sb[:], 1.0)

            # ---- phase 1: projections (+RoPE) ----
            with (
                tc.tile_pool(name="proj_sb", bufs=1) as proj_sb,
                tc.tile_pool(name="proj_ps", bufs=4, space="PSUM") as proj_ps,
                tc.tile_pool(name="vt_ps", bufs=2, space="PSUM") as vt_ps,
                tc.tile_pool(name="rope_tmp", bufs=2) as rtmp,
            ):
                xt_sb = proj_sb.tile([P, NJ, NO, QT], bf16)
                wqt_sb = proj_sb.tile([P, NO, GD], bf16)
                wkt_sb = proj_sb.tile([P, NO, HEAD_DIM], bf16)
                wvt_sb = proj_sb.tile([P, NO, HEAD_DIM], bf16)
                vt_sb = proj_sb.tile([P, S], bf16)

                # loads in dependency order; all fully-contiguous runs.
                # Tiny/strided DMAs (warmup seed, masks, ident) come AFTER the
                # first xt blocks — their small packets clog the single DMA
                # queue's FIFO ahead of the data the first matmuls need.
                # K proj only needs wkt + xt blocks; everything else (rope
                # tables, V/Q weights) is consumed later, so xt streams first.
                nc.sync.dma_start(wkt_sb[:], wkt_d[:])
                for j in range(NJ):
                    nc.sync.dma_start(xt_sb[:, j], xt_d[j])
                nc.sync.dma_start(cos_sb[:], cos_d[:])
                nc.sync.dma_start(sin_sb[:], sin_d[:])
                # warm the collective path (ncfw first-call + rank sync) early
                nc.sync.dma_start(wrm_in[:], ones_sb[:, 0:16])
                nc.gpsimd.collective_compute(
                    "AllToAll",
                    mybir.AluOpType.bypass,
                    replica_groups=ALL_CORES,
                    ins=[wrm_in.opt()],
                    outs=[wrm_out.opt()],
                )
                nc.sync.dma_start(wvt_sb[:], wvt_d[:])
                nc.sync.dma_start(msk_sb[:], msk_d.rearrange("r p q -> p r q"))
                nc.sync.dma_start(idn_sb[:], idn_d[:])
                nc.sync.dma_start(wqt_sb[:], wqt_d[:])
                nc.sync.dma_start(wot_sb[:], wot_d[:])

                def rope(dst, jsl):
                    # rotate-half form on deinterleaved rows:
                    #   rows 0:32 = a (even dims), 32:64 = b (odd dims)
                    #   new[0:64] = old[0:64]*cos64 + swap(old[0:64])*sin64
                    # with cos64 = [cosT; cosT], sin64 = [-sinT; sinT].
                    xs = rtmp.tile([64, QT], bf16, tag="xs")
                    nc.vector.tensor_copy(xs[0:32, :], dst[32:64, jsl])
                    nc.vector.tensor_copy(xs[32:64, :], dst[0:32, jsl])
                    t = rtmp.tile([64, QT], bf16, tag="t")
                    u = rtmp.tile([64, QT], bf16, tag="u")
                    nc.vector.tensor_mul(t[:], xs[:], sin_sb[:, jsl])
                    nc.vector.tensor_mul(u[:], dst[0:64, jsl], cos_sb[:, jsl])
                    nc.vector.tensor_add(dst[0:64, jsl], u[:], t[:])

                # kT = wkt.T @ xt  -> [128 dk, S]
                for j in range(NJ):
                    jsl = bass.ts(j, QT)
                    ps = proj_ps.tile([P, QT], f32, tag="ps")
                    for o in range(NO):
                        nc.tensor.matmul(
                            ps[:],
                            wkt_sb[:, o, :],
                            xt_sb[:, j, o, :],
                            start=(o == 0),
                            stop=(o == NO - 1),
                        )
                    nc.vector.tensor_copy(k_sb[:, jsl], ps[:])
                    rope(k_sb, jsl)

                # vT = wvt.T @ xt -> [128 dv, S]; then PE-transpose to v natural
                for j in range(NJ):
                    jsl = bass.ts(j, QT)
                    ps = proj_ps.tile([P, QT], f32, tag="ps")
                    for o in range(NO):
                        nc.tensor.matmul(
                            ps[:],
                            wvt_sb[:, o, :],
                            xt_sb[:, j, o, :],
                            start=(o == 0),
                            stop=(o == NO - 1),
                        )
                    nc.vector.tensor_copy(vt_sb[:, jsl], ps[:])
                    for cc in range(QT // P):
                        c = j * (QT // P) + cc
                        tp = vt_ps.tile([P, P], bf16, tag="vtp")
                        nc.tensor.transpose(tp[:], vt_sb[:, bass.ts(c, P)], idn_sb[:])
                        nc.scalar.copy(v_sb[:, c, :], tp[:])

                # qT per head -> [128 dq, S] x4
                for h in range(HG):
                    for j in range(NJ):
                        jsl = bass.ts(j, QT)
                        ps = proj_ps.tile([P, QT], f32, tag="ps")
                        for o in range(NO):
                            nc.tensor.matmul(
                                ps[:],
                                wqt_sb[:, o, bass.ts(h, P)],
                                xt_sb[:, j, o, :],
                                start=(o == 0),
                                stop=(o == NO - 1),
                            )
                        nc.vector.tensor_copy(q_sb[:, h, jsl], ps[:])
                        rope(q_sb[:, h, :], jsl)

            # ---- phase 2: attention (block-outer) + per-block AllToAll +
            #      interleaved out-projection ----
            with (
                tc.tile_pool(name="attn_sb", bufs=3) as attn_sb,
                tc.tile_pool(name="osb", bufs=2) as osb_pool,
                tc.tile_pool(name="og", bufs=2) as og_pool,
                tc.tile_pool(name="fin", bufs=2) as fin,
                tc.tile_pool(name="ps_sc", bufs=2, space="PSUM") as ps_sc,
                tc.tile_pool(name="ps_acc", bufs=2, space="PSUM") as ps_acc,
                tc.tile_pool(name="ps_out", bufs=2, space="PSUM") as ps_out,
            ):

                def outproj(jblk):
                    # 128 output rows of 512-row block jblk, full 2048-feature
                    # contraction, results DMA'd per n-slice.
                    # Column order: whole blocks og[.., 64b+r]; the split last
                    # block og[.., 64*half + 32b + r], computed in two
                    # 64-row passes so pass 0 runs while the second half's
                    # a2a is still in flight.
                    pieces = [
                        (i, row0, w)
                        for i, (row0, w) in enumerate(blocks)
                        if 512 * jblk <= row0 < 512 * (jblk + 1)
                    ]
                    split = len(pieces) > 1
                    og_sb = og_pool.tile([P, NO, P], bf16, tag="og")
                    ot = fin.tile([P, D_MODEL], f32, tag="ot")

                    def og_load(piece_idx):
                        i, _, w = pieces[piece_idx]
                        rb = w // N_CORES
                        src = a2a_out[i].rearrange(
                            "(b gfi p) r -> b p gfi r", b=B, p=P
                        )
                        for b in range(B):
                            c0 = 64 * piece_idx + rb * b if split else 64 * b
                            nc.sync.dma_start(og_sb[:, :, c0 : c0 + rb], src[b])

                    def mm_pass(r0, nrows):
                        for n in range(D_MODEL // QT):
                            ps = ps_out.tile([P, QT], f32, tag="pout")
                            for c in range(NO):
                                nc.tensor.matmul(
                                    ps[0:nrows, :],
                                    og_sb[:, c, r0 : r0 + nrows],
                                    wot_sb[:, c, bass.ts(n, QT)],
                                    start=(c == 0),
                                    stop=(c == NO - 1),
                                )
                            nsl = bass.ts(n, QT)
                            nc.scalar.copy(ot[r0 : r0 + nrows, nsl], ps[0:nrows, :])
                            nc.sync.dma_start(
                                out_d[128 * jblk + r0 : 128 * jblk + r0 + nrows, nsl],
                                ot[r0 : r0 + nrows, nsl],
                            )

                    if split:
                        og_load(0)
                        mm_pass(0, 64)
                        og_load(1)
                        mm_pass(64, 64)
                    else:
                        og_load(0)
                        mm_pass(0, P)

                done_oproj = 0
                for i, (row0, w) in enumerate(blocks):
                    nk = (row0 + w) // P  # causal: k chunks 0..nk-1
                    w2 = 2 * w
                    o_sb = osb_pool.tile([P, HG, QT], bf16, tag="osb")
                    for h in range(HG):
                        po = ps_acc.tile([P, QT], f32, tag="po")
                        acc = attn_sb.tile([P, QT], bf16, tag="acc")
                        for c2 in range(nk // 2):
                            sc = ps_sc.tile([P, 2 * QT], f32, tag="sc")
                            pt = attn_sb.tile([P, 2 * QT], bf16, tag="pt")
                            for t in (0, 1):
                                c = 2 * c2 + t
                                nc.tensor.matmul(
                                    sc[:, t * w : (t + 1) * w],
                                    k_sb[:, bass.ts(c, P)],
                                    q_sb[:, h, row0 : row0 + w],
                                    start=True,
                                    stop=True,
                                )
                            nc.scalar.activation(
                                pt[:, 0:w2],
                                sc[:, 0:w2],
                                mybir.ActivationFunctionType.Exp,
                                scale=scale,
                            )
                            for t in (0, 1):
                                c = 2 * c2 + t
                                r = c - row0 // P
                                if 0 <= r < w // P:
                                    nc.vector.tensor_mul(
                                        pt[:, t * w : (t + 1) * w],
                                        pt[:, t * w : (t + 1) * w],
                                        msk_sb[:, r, 0:w],
                                    )
                            # softmax-denominator partials accumulate on DVE
                            for t in (0, 1):
                                c = 2 * c2 + t
                                if c == 0:
                                    nc.vector.tensor_copy(
                                        acc[:, 0:w], pt[:, 0:w]
                                    )
                                else:
                                    nc.vector.tensor_add(
                                        acc[:, 0:w],
                                        acc[:, 0:w],
                                        pt[:, t * w : (t + 1) * w],
                                    )
                            for t in (0, 1):
                                c = 2 * c2 + t
                                nc.tensor.matmul(
                                    po[:, 0:w],
                                    v_sb[:, c, :],
                                    pt[:, t * w : (t + 1) * w],
                                    start=(c == 0),
                                    stop=(c == nk - 1),
                                )
                        # partition-reduce acc + broadcast in one matmul; the
                        # rs tile shares the out-proj PSUM pool so po can be
                        # double-buffered within the 8-bank budget
                        rs = ps_out.tile([P, QT], f32, tag="pout")
                        nc.tensor.matmul(
                            rs[:, 0:w], ones_sb[:], acc[:, 0:w], start=True, stop=True
                        )
                        rcp = attn_sb.tile([P, QT], f32, tag="rcp")
                        nc.vector.reciprocal_approx_fast(rcp[:, 0:w], rs[:, 0:w])
                        nc.vector.tensor_mul(o_sb[:, h, 0:w], po[:, 0:w], rcp[:, 0:w])

                    # ship block i: a2a_in[i][(d*4+h)*128+p, r] = o_sb[p,h,rb*d+r]
                    in_v = a2a_in[i].rearrange("(d h p) r -> p h d r", h=HG, p=P)
                    for h in range(HG):
                        nc.sync.dma_start(
                            in_v[:, h],
                            o_sb[:, h, 0:w].rearrange("p (d r) -> p d r", d=N_CORES),
                        )
                    nc.gpsimd.collective_compute(
                        "AllToAll",
                        mybir.AluOpType.bypass,
                        replica_groups=ALL_CORES,
                        ins=[a2a_in[i].opt()],
                        outs=[a2a_out[i].opt()],
                    )
                    # out-projection of a finished 512-row block overlaps this
                    # block's a2a
                    ready = row0 // 512  # 512-blocks fully shipped before i
                    if done_oproj < ready:
                        outproj(done_oproj)
                        done_oproj += 1
                while done_oproj < NJ:
                    outproj(done_oproj)
                    done_oproj += 1

    nc.compile()
    return nc


def host_prep(x, wq, wk, wv, wo, S):
    """Build the 8 per-core input maps (numpy, bf16)."""
    import ml_dtypes

    bf = ml_dtypes.bfloat16
    NJ = S // 512
    NO = D_MODEL // 128
    perm = np.concatenate(
        [np.arange(0, ROPE, 2), np.arange(1, ROPE, 2), np.arange(ROPE, HEAD_DIM)]
    )
    wq_p = wq.reshape(N_HEADS, HEAD_DIM, D_MODEL)[:, perm, :]
    wk_p = wk.reshape(N_KV, HEAD_DIM, D_MODEL)[:, perm, :]

    inv = THETA ** (-np.arange(0, ROPE, 2, dtype=np.float64) / ROPE)  # [32]
    t = np.arange(S, dtype=np.float64)
    ang = np.outer(inv, t)  # [32, S]
    cosT, sinT = np.cos(ang), np.sin(ang)
    cos = np.ascontiguousarray(np.concatenate([cosT, cosT], 0)).astype(bf)  # [64,S]
    sin = np.ascontiguousarray(np.concatenate([-sinT, sinT], 0)).astype(bf)  # [64,S]

    # causal masks for diagonal tiles: r = k_chunk - row0/128 in [0,4)
    kk = np.arange(128)[:, None]
    qq = np.arange(512)[None, :]
    masks = np.stack(
        [(128 * r + kk <= qq) for r in range(4)]
    ).astype(bf)  # [4,128,512]

    ident = np.eye(128, dtype=np.float32).astype(bf)

    def chunked(wT):  # [D, M] -> [128, NO, M] contiguous
        return np.ascontiguousarray(
            wT.reshape(NO, 128, wT.shape[1]).transpose(1, 0, 2)
        ).astype(bf)

    wot = chunked(wo.T.astype(np.float32))

    in_maps = []
    for c in range(N_CORES):
        b, g = divmod(c, 4)
        # xt[j, p, o, s] = x[b, 512j+s, 128o+p]
        xt = np.ascontiguousarray(
            x[b, :S].reshape(NJ, 512, NO, 128).transpose(0, 3, 2, 1)
        ).astype(bf)
        wqt = chunked(wq_p[HG * g : HG * (g + 1)].reshape(GD, D_MODEL).T)
        wkt = chunked(wk_p[g].T)
        wvt = chunked(wv[HEAD_DIM * g : HEAD_DIM * (g + 1)].T)
        in_maps.append(
            {
                "xt": xt,
                "wqt": wqt,
                "wkt": wkt,
                "wvt": wvt,
                "wot": wot,
                "cos": cos,
                "sin": sin,
                "masks": masks,
                "ident": ident,
            }
        )
    return in_maps


def core_rows(S, d):
    """Device out_d row index -> (batch, seq) for core d, in device order."""
    rows = []
    for jblk in range(S // 512):
        row0 = 512 * jblk
        if jblk < S // 512 - 1:
            for b in range(B):
                rows += [(b, row0 + 64 * d + r) for r in range(64)]
        else:
            for half in range(2):
                for b in range(B):
                    rows += [
                        (b, row0 + 256 * half + 32 * d + r) for r in range(32)
                    ]
    return rows


def run(x, wq, wk, wv, wo, S=None, trace=False):
    from concourse.bass_utils import run_bass_kernel_spmd

    if S is None:
        S = x.shape[1]
    if S not in _BUILD_CACHE:
        _BUILD_CACHE[S] = build_kernel(S)
    nc = _BUILD_CACHE[S]
    in_maps = host_prep(x, wq, wk, wv, wo, S)
    res = run_bass_kernel_spmd(nc, in_maps, core_ids=list(range(N_CORES)), trace=trace)
    out = np.empty((B, S, D_MODEL), np.float32)
    for d in range(N_CORES):
        o = res.results[d]["out"]  # [S//4, D]
        rows = core_rows(S, d)
        bs = np.array([r[0] for r in rows])
        ss = np.array([r[1] for r in rows])
        out[bs, ss, :] = o
    return out, res


def kernel(x, wq, wk, wv, wo):
    x = np.asarray(x, np.float32)
    wq = np.asarray(wq, np.float32)
    wk = np.asarray(wk, np.float32)
    wv = np.asarray(wv, np.float32)
    wo = np.asarray(wo, np.float32)
    out, _ = run(x, wq, wk, wv, wo)
    return out


# revision 32
# speedup vs baseline: 1.4625x; 1.0076x over previous
"""GroupedQueryAttention Trainium2 kernel (8 NeuronCores, SPMD).

Sharding: core c -> (batch b = c // 4, kv-group g = c % 4).
Each core computes q/k/v projections for its 4 query heads + 1 kv head,
partial-RoPE, and causal attention for its heads over the full sequence.
Attention outputs are resharded with 8-way AllToAlls (one per seq block,
fired as soon as that block's heads finish, overlapped with later
blocks): for each block, core d ends up owning width/8 rows of BOTH
batches with all 16 heads' features, then runs the full out-projection
for those rows (full wo on every core, no reduction needed).

Blocks: three 512-row blocks + two 256-row half-blocks at the end, so
the final (serial) a2a + out-projection tail is halved. A tiny dummy
AllToAll at kernel start warms the collective path (first-call firmware
overhead + rank sync) off the critical path.

All device matmuls run in bf16 (fp32 PSUM accumulation). The host
pre-chunks operands so every load is a single fully-contiguous DMA and
the contraction dim lands on SBUF partitions (V is transposed on the PE
array):
  xt   [NJ, 128, 16, 512]  x[b].T per (block, partition, chunk)
  wqt  [128, 16, 512]      perm(wq)[group].T chunked (RoPE-deinterleaved)
  wkt  [128, 16, 128]      perm(wk)[group].T chunked
  wvt  [128, 16, 128]      wv[group].T chunked
  wot  [128, 16, 2048]     full wo.T chunked (same on every core)
The RoPE deinterleave permutation reorders each head's first 64 dims to
[evens, odds]; since q and k use the same permutation, q.k dot products
are unchanged and it never needs undoing.
"""

import math
import sys

sys.path.insert(0, "/opt/trn_rl_repo")

import numpy as np  # noqa: E402

D_MODEL = 2048
N_HEADS = 16
N_KV = 4
HEAD_DIM = 128
ROPE = 64
THETA = 10000.0
B = 2
HG = N_HEADS // N_KV  # 4 query heads per kv group
GD = HG * HEAD_DIM  # 512 o-features per group
N_CORES = 8
ALL_CORES = [list(range(N_CORES))]

_BUILD_CACHE: dict = {}


def seq_blocks(S):
    """(row0, width) attention/a2a blocks: 512-wide, last split in two."""
    blocks = [(r, 512) for r in range(0, S - 512, 512)]
    blocks += [(S - 512, 256), (S - 256, 256)]
    return blocks


def build_kernel(S: int):
    """Build the per-core Bass program for sequence length S (multiple of 512)."""
    import concourse.bass as bass
    import concourse.mybir as mybir
    import concourse.tile as tile
    from concourse import bacc

    assert S % 512 == 0
    P = 128
    QT = 512  # max q tile (free dim of scoresT)
    NJ = S // QT  # 512-row seq blocks
    NO = D_MODEL // P  # contraction chunks for projections (16)
    NS = S // P  # seq chunks of 128
    bf16 = mybir.dt.bfloat16
    f32 = mybir.dt.float32
    f8 = mybir.dt.float8e4
    scale = 1.0 / math.sqrt(HEAD_DIM)
    blocks = seq_blocks(S)

    nc = bacc.Bacc(None, target_bir_lowering=False, debug=False, num_devices=N_CORES)

    xt_d = nc.declare_dram_parameter("xt", [NJ, P, NO, QT], bf16, isOutput=False)
    wqt_d = nc.declare_dram_parameter("wqt", [P, NO, GD], bf16, isOutput=False)
    wkt_d = nc.declare_dram_parameter("wkt", [P, NO, HEAD_DIM], bf16, isOutput=False)
    wvt_d = nc.declare_dram_parameter("wvt", [P, NO, HEAD_DIM], bf16, isOutput=False)
    wot_d = nc.declare_dram_parameter("wot", [P, NO, D_MODEL], bf16, isOutput=False)
    cos_d = nc.declare_dram_parameter("cos", [ROPE, S], bf16, isOutput=False)
    sin_d = nc.declare_dram_parameter("sin", [ROPE, S], bf16, isOutput=False)
    msk_d = nc.declare_dram_parameter("masks", [4, P, QT], bf16, isOutput=False)
    idn_d = nc.declare_dram_parameter("ident", [P, P], bf16, isOutput=False)
    # per-core output rows; see host gather for the row mapping
    out_d = nc.declare_dram_parameter("out", [S // 4, D_MODEL], f32, isOutput=True)

    with tile.TileContext(nc) as tc:
        with (
            tc.tile_pool(name="persist", bufs=1) as persist,
            tc.tile_pool(name="dram", bufs=1, space="DRAM") as dram,
        ):
            # ---- persistent SBUF state ----
            q_sb = persist.tile([P, HG, S], bf16)  # qT, per-head chunks
            k_sb = persist.tile([P, S], bf16)  # kT
            v_sb = persist.tile([P, NS, HEAD_DIM], bf16)  # v natural
            # (fp8 DoubleRow PV was tried and reverted: e4m3's ~3.6% rms
            # quantization hits the softmax-weighted mean unattenuated,
            # measured 3.5e-2 rel err vs the 2e-2 budget)
            cos_sb = persist.tile([ROPE, S], bf16)
            sin_sb = persist.tile([ROPE, S], bf16)
            msk_sb = persist.tile([P, 4, QT], bf16)
            ones_sb = persist.tile([P, P], bf16)
            idn_sb = persist.tile([P, P], bf16)
            wot_sb = persist.tile([P, NO, D_MODEL], bf16)  # full wo.T, chunked

            a2a_in = [
                dram.tile([N_CORES * GD, w // N_CORES], bf16, name=f"a2ain{i}")
                for i, (_, w) in enumerate(blocks)
            ]
            a2a_out = [
                dram.tile([N_CORES * GD, w // N_CORES], bf16, name=f"a2aout{i}")
                for i, (_, w) in enumerate(blocks)
            ]
            wrm_in = dram.tile([P, 16], bf16)
            wrm_out = dram.tile([P, 16], bf16)

            nc.vector.memset(ones_sb[:], 1.0)

            # ---- phase 1: projections (+RoPE) ----
            with (
                tc.tile_pool(name="proj_sb", bufs=1) as proj_sb,
                tc.tile_pool(name="proj_ps", bufs=4, space="PSUM") as proj_ps,
                tc.tile_pool(name="vt_ps", bufs=2, space="PSUM") as vt_ps,
                tc.tile_pool(name="rope_tmp", bufs=2) as rtmp,
            ):
                xt_sb = proj_sb.tile([P, NJ, NO, QT], bf16)
                wqt_sb = proj_sb.tile([P, NO, GD], bf16)
                wkt_sb = proj_sb.tile([P, NO, HEAD_DIM], bf16)
                wvt_sb = proj_sb.tile([P, NO, HEAD_DIM], bf16)
                vt_sb = proj_sb.tile([P, S], bf16)

                # loads in dependency order; all fully-contiguous runs.
                # Tiny/strided DMAs (warmup seed, masks, ident) come AFTER the
                # first xt blocks — their small packets clog the single DMA
                # queue's FIFO ahead of the data the first matmuls need.
                # K proj only needs wkt + xt blocks; everything else (rope
                # tables, V/Q weights) is consumed later, so xt streams first.
                nc.sync.dma_start(wkt_sb[:], wkt_d[:])
                for j in range(NJ):
                    nc.sync.dma_start(xt_sb[:, j], xt_d[j])
                nc.sync.dma_start(cos_sb[:], cos_d[:])
                nc.sync.dma_start(sin_sb[:], sin_d[:])
                # warm the collective path (ncfw first-call + rank sync) early
                nc.sync.dma_start(wrm_in[:], ones_sb[:, 0:16])
                nc.gpsimd.collective_compute(
                    "AllToAll",
                    mybir.AluOpType.bypass,
                    replica_groups=ALL_CORES,
                    ins=[wrm_in.opt()],
                    outs=[wrm_out.opt()],
                )
                nc.sync.dma_start(wvt_sb[:], wvt_d[:])
                nc.sync.dma_start(msk_sb[:], msk_d.rearrange("r p q -> p r q"))
                nc.sync.dma_start(idn_sb[:], idn_d[:])
                nc.sync.dma_start(wqt_sb[:], wqt_d[:])
                nc.sync.dma_start(wot_sb[:], wot_d[:])

                def rope(dst, jsl):
                    # rotate-half form on deinterleaved rows:
                    #   rows 0:32 = a (even dims), 32:64 = b (odd dims)
                    #   new[0:64] = old[0:64]*cos64 + swap(old[0:64])*sin64
                    # with cos64 = [cosT; cosT], sin64 = [-sinT; sinT].
                    xs = rtmp.tile([64, QT], bf16, tag="xs")
                    nc.vector.tensor_copy(xs[0:32, :], dst[32:64, jsl])
                    nc.vector.tensor_copy(xs[32:64, :], dst[0:32, jsl])
                    t = rtmp.tile([64, QT], bf16, tag="t")
                    u = rtmp.tile([64, QT], bf16, tag="u")
                    nc.vector.tensor_mul(t[:], xs[:], sin_sb[:, jsl])
                    nc.vector.tensor_mul(u[:], dst[0:64, jsl], cos_sb[:, jsl])
                    nc.vector.tensor_add(dst[0:64, jsl], u[:], t[:])

                # kT = wkt.T @ xt  -> [128 dk, S]
                for j in range(NJ):
                    jsl = bass.ts(j, QT)
                    ps = proj_ps.tile([P, QT], f32, tag="ps")
                    for o in range(NO):
                        nc.tensor.matmul(
                            ps[:],
                            wkt_sb[:, o, :],
                            xt_sb[:, j, o, :],
                            start=(o == 0),
                            stop=(o == NO - 1),
                        )
                    nc.vector.tensor_copy(k_sb[:, jsl], ps[:])
                    rope(k_sb, jsl)

                # vT = wvt.T @ xt -> [128 dv, S]; then PE-transpose to v natural
                for j in range(NJ):
                    jsl = bass.ts(j, QT)
                    ps = proj_ps.tile([P, QT], f32, tag="ps")
                    for o in range(NO):
                        nc.tensor.matmul(
                            ps[:],
                            wvt_sb[:, o, :],
                            xt_sb[:, j, o, :],
                            start=(o == 0),
                            stop=(o == NO - 1),
                        )
                    nc.vector.tensor_copy(vt_sb[:, jsl], ps[:])
                    for cc in range(QT // P):
                        c = j * (QT // P) + cc
                        tp = vt_ps.tile([P, P], bf16, tag="vtp")
                        nc.tensor.transpose(tp[:], vt_sb[:, bass.ts(c, P)], idn_sb[:])
                        nc.scalar.copy(v_sb[:, c, :], tp[:])

                # qT per head -> [128 dq, S] x4
                for h in range(HG):
                    for j in range(NJ):
                        jsl = bass.ts(j, QT)
                        ps = proj_ps.tile([P, QT], f32, tag="ps")
                        for o in range(NO):
                            nc.tensor.matmul(
                                ps[:],
                                wqt_sb[:, o, bass.ts(h, P)],
                                xt_sb[:, j, o, :],
                                start=(o == 0),
                                stop=(o == NO - 1),
                            )
                        nc.vector.tensor_copy(q_sb[:, h, jsl], ps[:])
                        rope(q_sb[:, h, :], jsl)

            # ---- phase 2: attention (block-outer) + per-block AllToAll +
            #      interleaved out-projection ----
            with (
                tc.tile_pool(name="attn_sb", bufs=3) as attn_sb,
                tc.tile_pool(name="osb", bufs=2) as osb_pool,
                tc.tile_pool(name="og", bufs=2) as og_pool,
                tc.tile_pool(name="fin", bufs=2) as fin,
                tc.tile_pool(name="ps_sc", bufs=2, space="PSUM") as ps_sc,
                tc.tile_pool(name="ps_acc", bufs=2, space="PSUM") as ps_acc,
                tc.tile_pool(name="ps_out", bufs=2, space="PSUM") as ps_out,
            ):

                def outproj(jblk):
                    # 128 output rows of 512-row block jblk, full 2048-feature
                    # contraction, results DMA'd per n-slice.
                    # Column order: whole blocks og[.., 64b+r]; the split last
                    # block og[.., 64*half + 32b + r], computed in two
                    # 64-row passes so pass 0 runs while the second half's
                    # a2a is still in flight.
                    pieces = [
                        (i, row0, w)
                        for i, (row0, w) in enumerate(blocks)
                        if 512 * jblk <= row0 < 512 * (jblk + 1)
                    ]
                    split = len(pieces) > 1
                    og_sb = og_pool.tile([P, NO, P], bf16, tag="og")
                    ot = fin.tile([P, D_MODEL], f32, tag="ot")

                    def og_load(piece_idx):
                        i, _, w = pieces[piece_idx]
                        rb = w // N_CORES
                        src = a2a_out[i].rearrange(
                            "(b gfi p) r -> b p gfi r", b=B, p=P
                        )
                        for b in range(B):
                            c0 = 64 * piece_idx + rb * b if split else 64 * b
                            nc.sync.dma_start(og_sb[:, :, c0 : c0 + rb], src[b])

                    def mm_pass(r0, nrows):
                        for n in range(D_MODEL // QT):
                            ps = ps_out.tile([P, QT], f32, tag="pout")
                            for c in range(NO):
                                nc.tensor.matmul(
                                    ps[0:nrows, :],
                                    og_sb[:, c, r0 : r0 + nrows],
                                    wot_sb[:, c, bass.ts(n, QT)],
                                    start=(c == 0),
                                    stop=(c == NO - 1),
                                )
                            nsl = bass.ts(n, QT)
                            nc.scalar.copy(ot[r0 : r0 + nrows, nsl], ps[0:nrows, :])
                            nc.sync.dma_start(
                                out_d[128 * jblk + r0 : 128 * jblk + r0 + nrows, nsl],
                                ot[r0 : r0 + nrows, nsl],
                            )

                    if split:
                        og_load(0)
                        mm_pass(0, 64)
                        og_load(1)
                        mm_pass(64, 64)
                    else:
                        og_load(0)
                        mm_pass(0, P)

                # out-proj of block J is emitted two a2a's after J's own, so
                # each a2a gets ~2 attention blocks of slack before its data
                # is needed (the collectives re-sync all 8 cores, and the
                # slowest core can lag by ~25us)
                nb = len(blocks)
                ready_after = [max(0, i - 1) for i in range(nb)]
                ready_after[nb - 2] = NJ - 1
                ready_after[nb - 1] = NJ
                done_oproj = 0
                for i, (row0, w) in enumerate(blocks):
                    nk = (row0 + w) // P  # causal: k chunks 0..nk-1
                    w2 = 2 * w
                    o_sb = osb_pool.tile([P, HG, QT], bf16, tag="osb")
                    for h in range(HG):
                        po = ps_acc.tile([P, QT], f32, tag="po")
                        acc = attn_sb.tile([P, QT], bf16, tag="acc")
                        for c2 in range(nk // 2):
                            sc = ps_sc.tile([P, 2 * QT], f32, tag="sc")
                            pt = attn_sb.tile([P, 2 * QT], bf16, tag="pt")
                            for t in (0, 1):
                                c = 2 * c2 + t
                                nc.tensor.matmul(
                                    sc[:, t * w : (t + 1) * w],
                                    k_sb[:, bass.ts(c, P)],
                                    q_sb[:, h, row0 : row0 + w],
                                    start=True,
                                    stop=True,
                                )
                            nc.scalar.activation(
                                pt[:, 0:w2],
                                sc[:, 0:w2],
                                mybir.ActivationFunctionType.Exp,
                                scale=scale,
                            )
                            for t in (0, 1):
                                c = 2 * c2 + t
                                r = c - row0 // P
                                if 0 <= r < w // P:
                                    nc.vector.tensor_mul(
                                        pt[:, t * w : (t + 1) * w],
                                        pt[:, t * w : (t + 1) * w],
                                        msk_sb[:, r, 0:w],
                                    )
                            # softmax-denominator partials accumulate on DVE
                            for t in (0, 1):
                                c = 2 * c2 + t
                                if c == 0:
                                    nc.vector.tensor_copy(
                                        acc[:, 0:w], pt[:, 0:w]
                                    )
                                else:
                                    nc.vector.tensor_add(
                                        acc[:, 0:w],
                                        acc[:, 0:w],
                                        pt[:, t * w : (t + 1) * w],
                                    )
                            for t in (0, 1):
                                c = 2 * c2 + t
                                nc.tensor.matmul(
                                    po[:, 0:w],
                                    v_sb[:, c, :],
                                    pt[:, t * w : (t + 1) * w],
                                    start=(c == 0),
                                    stop=(c == nk - 1),
                                )
                        # partition-reduce acc + broadcast in one matmul; the
                        # rs tile shares the out-proj PSUM pool so po can be
                        # double-buffered within the 8-bank budget
                        rs = ps_out.tile([P, QT], f32, tag="pout")
                        nc.tensor.matmul(
                            rs[:, 0:w], ones_sb[:], acc[:, 0:w], start=True, stop=True
                        )
                        rcp = attn_sb.tile([P, QT], f32, tag="rcp")
                        nc.vector.reciprocal_approx_fast(rcp[:, 0:w], rs[:, 0:w])
                        nc.vector.tensor_mul(o_sb[:, h, 0:w], po[:, 0:w], rcp[:, 0:w])

                    # ship block i: a2a_in[i][(d*4+h)*128+p, r] = o_sb[p,h,rb*d+r]
                    in_v = a2a_in[i].rearrange("(d h p) r -> p h d r", h=HG, p=P)
                    for h in range(HG):
                        nc.sync.dma_start(
                            in_v[:, h],
                            o_sb[:, h, 0:w].rearrange("p (d r) -> p d r", d=N_CORES),
                        )
                    nc.gpsimd.collective_compute(
                        "AllToAll",
                        mybir.AluOpType.bypass,
                        replica_groups=ALL_CORES,
                        ins=[a2a_in[i].opt()],
                        outs=[a2a_out[i].opt()],
                    )
                    while done_oproj < ready_after[i]:
                        outproj(done_oproj)
                        done_oproj += 1
                while done_oproj < NJ:
                    outproj(done_oproj)
                    done_oproj += 1

    nc.compile()
    return nc


def host_prep(x, wq, wk, wv, wo, S):
    """Build the 8 per-core input maps (numpy, bf16)."""
    import ml_dtypes

    bf = ml_dtypes.bfloat16
    NJ = S // 512
    NO = D_MODEL // 128
    perm = np.concatenate(
        [np.arange(0, ROPE, 2), np.arange(1, ROPE, 2), np.arange(ROPE, HEAD_DIM)]
    )
    wq_p = wq.reshape(N_HEADS, HEAD_DIM, D_MODEL)[:, perm, :]
    wk_p = wk.reshape(N_KV, HEAD_DIM, D_MODEL)[:, perm, :]

    inv = THETA ** (-np.arange(0, ROPE, 2, dtype=np.float64) / ROPE)  # [32]
    t = np.arange(S, dtype=np.float64)
    ang = np.outer(inv, t)  # [32, S]
    cosT, sinT = np.cos(ang), np.sin(ang)
    cos = np.ascontiguousarray(np.concatenate([cosT, cosT], 0)).astype(bf)  # [64,S]
    sin = np.ascontiguousarray(np.concatenate([-sinT, sinT], 0)).astype(bf)  # [64,S]

    # causal masks for diagonal tiles: r = k_chunk - row0/128 in [0,4)
    kk = np.arange(128)[:, None]
    qq = np.arange(512)[None, :]
    masks = np.stack(
        [(128 * r + kk <= qq) for r in range(4)]
    ).astype(bf)  # [4,128,512]

    ident = np.eye(128, dtype=np.float32).astype(bf)

    def chunked(wT):  # [D, M] -> [128, NO, M] contiguous
        return np.ascontiguousarray(
            wT.reshape(NO, 128, wT.shape[1]).transpose(1, 0, 2)
        ).astype(bf)

    wot = chunked(wo.T.astype(np.float32))

    in_maps = []
    for c in range(N_CORES):
        b, g = divmod(c, 4)
        # xt[j, p, o, s] = x[b, 512j+s, 128o+p]
        xt = np.ascontiguousarray(
            x[b, :S].reshape(NJ, 512, NO, 128).transpose(0, 3, 2, 1)
        ).astype(bf)
        wqt = chunked(wq_p[HG * g : HG * (g + 1)].reshape(GD, D_MODEL).T)
        wkt = chunked(wk_p[g].T)
        wvt = chunked(wv[HEAD_DIM * g : HEAD_DIM * (g + 1)].T)
        in_maps.append(
            {
                "xt": xt,
                "wqt": wqt,
                "wkt": wkt,
                "wvt": wvt,
                "wot": wot,
                "cos": cos,
                "sin": sin,
                "masks": masks,
                "ident": ident,
            }
        )
    return in_maps


def core_rows(S, d):
    """Device out_d row index -> (batch, seq) for core d, in device order."""
    rows = []
    for jblk in range(S // 512):
        row0 = 512 * jblk
        if jblk < S // 512 - 1:
            for b in range(B):
                rows += [(b, row0 + 64 * d + r) for r in range(64)]
        else:
            for half in range(2):
                for b in range(B):
                    rows += [
                        (b, row0 + 256 * half + 32 * d + r) for r in range(32)
                    ]
    return rows


def run(x, wq, wk, wv, wo, S=None, trace=False):
    from concourse.bass_utils import run_bass_kernel_spmd

    if S is None:
        S = x.shape[1]
    if S not in _BUILD_CACHE:
        _BUILD_CACHE[S] = build_kernel(S)
    nc = _BUILD_CACHE[S]
    in_maps = host_prep(x, wq, wk, wv, wo, S)
    res = run_bass_kernel_spmd(nc, in_maps, core_ids=list(range(N_CORES)), trace=trace)
    out = np.empty((B, S, D_MODEL), np.float32)
    for d in range(N_CORES):
        o = res.results[d]["out"]  # [S//4, D]
        rows = core_rows(S, d)
        bs = np.array([r[0] for r in rows])
        ss = np.array([r[1] for r in rows])
        out[bs, ss, :] = o
    return out, res


def kernel(x, wq, wk, wv, wo):
    x = np.asarray(x, np.float32)
    wq = np.asarray(wq, np.float32)
    wk = np.asarray(wk, np.float32)
    wv = np.asarray(wv, np.float32)
    wo = np.asarray(wo, np.float32)
    out, _ = run(x, wq, wk, wv, wo)
    return out


# revision 35
# speedup vs baseline: 1.5036x; 1.0281x over previous
"""GroupedQueryAttention Trainium2 kernel (8 NeuronCores, SPMD).

Sharding: core c -> (batch b = c // 4, kv-group g = c % 4).
Each core computes q/k/v projections for its 4 query heads + 1 kv head,
partial-RoPE, and causal attention for its heads over the full sequence.
Attention outputs are resharded with 8-way AllToAlls (one per seq block,
fired as soon as that block's heads finish, overlapped with later
blocks): for each block, core d ends up owning width/8 rows of BOTH
batches with all 16 heads' features, then runs the full out-projection
for those rows (full wo on every core, no reduction needed).

Blocks: three 512-row blocks + two 256-row half-blocks at the end, so
the final (serial) a2a + out-projection tail is halved. A tiny dummy
AllToAll at kernel start warms the collective path (first-call firmware
overhead + rank sync) off the critical path.

All device matmuls run in bf16 (fp32 PSUM accumulation). The host
pre-chunks operands so every load is a single fully-contiguous DMA and
the contraction dim lands on SBUF partitions (V is transposed on the PE
array):
  xt   [NJ, 128, 16, 512]  x[b].T per (block, partition, chunk)
  wqt  [128, 16, 512]      perm(wq)[group].T chunked (RoPE-deinterleaved)
  wkt  [128, 16, 128]      perm(wk)[group].T chunked
  wvt  [128, 16, 128]      wv[group].T chunked
  wot  [128, 16, 2048]     full wo.T chunked (same on every core)
The RoPE deinterleave permutation reorders each head's first 64 dims to
[evens, odds]; since q and k use the same permutation, q.k dot products
are unchanged and it never needs undoing.
"""

import math
import sys

sys.path.insert(0, "/opt/trn_rl_repo")

import numpy as np  # noqa: E402

D_MODEL = 2048
N_HEADS = 16
N_KV = 4
HEAD_DIM = 128
ROPE = 64
THETA = 10000.0
B = 2
HG = N_HEADS // N_KV  # 4 query heads per kv group
GD = HG * HEAD_DIM  # 512 o-features per group
N_CORES = 8
ALL_CORES = [list(range(N_CORES))]

_BUILD_CACHE: dict = {}


def seq_blocks(S):
    """(row0, width) attention/a2a blocks: 512-wide, last split in two."""
    blocks = [(r, 512) for r in range(0, S - 512, 512)]
    blocks += [(S - 512, 256), (S - 256, 256)]
    return blocks


def build_kernel(S: int):
    """Build the per-core Bass program for sequence length S (multiple of 512)."""
    import concourse.bass as bass
    import concourse.mybir as mybir
    import concourse.tile as tile
    from concourse import bacc

    assert S % 512 == 0
    P = 128
    QT = 512  # max q tile (free dim of scoresT)
    NJ = S // QT  # 512-row seq blocks
    NO = D_MODEL // P  # contraction chunks for projections (16)
    NS = S // P  # seq chunks of 128
    bf16 = mybir.dt.bfloat16
    f32 = mybir.dt.float32
    f8 = mybir.dt.float8e4
    scale = 1.0 / math.sqrt(HEAD_DIM)
    blocks = seq_blocks(S)

    nc = bacc.Bacc(None, target_bir_lowering=False, debug=False, num_devices=N_CORES)

    xt_d = nc.declare_dram_parameter("xt", [NJ, P, NO, QT], bf16, isOutput=False)
    wqt_d = nc.declare_dram_parameter("wqt", [P, NO, GD], bf16, isOutput=False)
    wkt_d = nc.declare_dram_parameter("wkt", [P, NO, HEAD_DIM], bf16, isOutput=False)
    wvt_d = nc.declare_dram_parameter("wvt", [P, NO, HEAD_DIM], bf16, isOutput=False)
    wot_d = nc.declare_dram_parameter("wot", [P, NO, D_MODEL], bf16, isOutput=False)
    cos_d = nc.declare_dram_parameter("cos", [ROPE, S], bf16, isOutput=False)
    sin_d = nc.declare_dram_parameter("sin", [ROPE, S], bf16, isOutput=False)
    msk_d = nc.declare_dram_parameter("masks", [4, P, QT], bf16, isOutput=False)
    idn_d = nc.declare_dram_parameter("ident", [P, P], bf16, isOutput=False)
    # per-core output rows; see host gather for the row mapping (bf16, host
    # converts to f32 — halves the 4MB/core output DMA)
    out_d = nc.declare_dram_parameter("out", [S // 4, D_MODEL], bf16, isOutput=True)

    with tile.TileContext(nc) as tc:
        with (
            tc.tile_pool(name="persist", bufs=1) as persist,
            tc.tile_pool(name="dram", bufs=1, space="DRAM") as dram,
        ):
            # ---- persistent SBUF state ----
            q_sb = persist.tile([P, HG, S], bf16)  # qT, per-head chunks
            k_sb = persist.tile([P, S], bf16)  # kT
            v_sb = persist.tile([P, NS, HEAD_DIM], bf16)  # v natural
            # (fp8 DoubleRow PV was tried and reverted: e4m3's ~3.6% rms
            # quantization hits the softmax-weighted mean unattenuated,
            # measured 3.5e-2 rel err vs the 2e-2 budget)
            cos_sb = persist.tile([ROPE, S], bf16)
            sin_sb = persist.tile([ROPE, S], bf16)
            msk_sb = persist.tile([P, 4, QT], bf16)
            ones_sb = persist.tile([P, P], bf16)
            idn_sb = persist.tile([P, P], bf16)
            wot_sb = persist.tile([P, NO, D_MODEL], bf16)  # full wo.T, chunked

            a2a_in = [
                dram.tile([N_CORES * GD, w // N_CORES], bf16, name=f"a2ain{i}")
                for i, (_, w) in enumerate(blocks)
            ]
            a2a_out = [
                dram.tile([N_CORES * GD, w // N_CORES], bf16, name=f"a2aout{i}")
                for i, (_, w) in enumerate(blocks)
            ]
            wrm_in = dram.tile([P, 16], bf16)
            wrm_out = dram.tile([P, 16], bf16)

            nc.vector.memset(ones_sb[:], 1.0)

            # ---- phase 1: projections (+RoPE) ----
            with (
                tc.tile_pool(name="proj_sb", bufs=1) as proj_sb,
                tc.tile_pool(name="proj_ps", bufs=4, space="PSUM") as proj_ps,
                tc.tile_pool(name="vt_ps", bufs=2, space="PSUM") as vt_ps,
                tc.tile_pool(name="rope_tmp", bufs=2) as rtmp,
            ):
                xt_sb = proj_sb.tile([P, NJ, NO, QT], bf16)
                wqt_sb = proj_sb.tile([P, NO, GD], bf16)
                wkt_sb = proj_sb.tile([P, NO, HEAD_DIM], bf16)
                wvt_sb = proj_sb.tile([P, NO, HEAD_DIM], bf16)
                vt_sb = proj_sb.tile([P, S], bf16)

                # loads in dependency order; all fully-contiguous runs.
                # Tiny/strided DMAs (warmup seed, masks, ident) come AFTER the
                # first xt blocks — their small packets clog the single DMA
                # queue's FIFO ahead of the data the first matmuls need.
                # K proj only needs wkt + xt blocks; everything else (rope
                # tables, V/Q weights) is consumed later, so xt streams first.
                nc.sync.dma_start(wkt_sb[:], wkt_d[:])
                for j in range(NJ):
                    nc.sync.dma_start(xt_sb[:, j], xt_d[j])
                nc.sync.dma_start(cos_sb[:], cos_d[:])
                nc.sync.dma_start(sin_sb[:], sin_d[:])
                # warm the collective path (ncfw first-call + rank sync) early
                nc.sync.dma_start(wrm_in[:], ones_sb[:, 0:16])
                nc.gpsimd.collective_compute(
                    "AllToAll",
                    mybir.AluOpType.bypass,
                    replica_groups=ALL_CORES,
                    ins=[wrm_in.opt()],
                    outs=[wrm_out.opt()],
                )
                nc.sync.dma_start(wvt_sb[:], wvt_d[:])
                nc.sync.dma_start(msk_sb[:], msk_d.rearrange("r p q -> p r q"))
                nc.sync.dma_start(idn_sb[:], idn_d[:])
                nc.sync.dma_start(wqt_sb[:], wqt_d[:])
                nc.sync.dma_start(wot_sb[:], wot_d[:])

                def rope(dst, jsl):
                    # rotate-half form on deinterleaved rows:
                    #   rows 0:32 = a (even dims), 32:64 = b (odd dims)
                    #   new[0:64] = old[0:64]*cos64 + swap(old[0:64])*sin64
                    # with cos64 = [cosT; cosT], sin64 = [-sinT; sinT].
                    xs = rtmp.tile([64, QT], bf16, tag="xs")
                    nc.vector.tensor_copy(xs[0:32, :], dst[32:64, jsl])
                    nc.vector.tensor_copy(xs[32:64, :], dst[0:32, jsl])
                    t = rtmp.tile([64, QT], bf16, tag="t")
                    u = rtmp.tile([64, QT], bf16, tag="u")
                    nc.vector.tensor_mul(t[:], xs[:], sin_sb[:, jsl])
                    nc.vector.tensor_mul(u[:], dst[0:64, jsl], cos_sb[:, jsl])
                    nc.vector.tensor_add(dst[0:64, jsl], u[:], t[:])

                # kT = wkt.T @ xt  -> [128 dk, S]
                for j in range(NJ):
                    jsl = bass.ts(j, QT)
                    ps = proj_ps.tile([P, QT], f32, tag="ps")
                    for o in range(NO):
                        nc.tensor.matmul(
                            ps[:],
                            wkt_sb[:, o, :],
                            xt_sb[:, j, o, :],
                            start=(o == 0),
                            stop=(o == NO - 1),
                        )
                    nc.vector.tensor_copy(k_sb[:, jsl], ps[:])
                    rope(k_sb, jsl)

                # vT = wvt.T @ xt -> [128 dv, S]; then PE-transpose to v natural
                for j in range(NJ):
                    jsl = bass.ts(j, QT)
                    ps = proj_ps.tile([P, QT], f32, tag="ps")
                    for o in range(NO):
                        nc.tensor.matmul(
                            ps[:],
                            wvt_sb[:, o, :],
                            xt_sb[:, j, o, :],
                            start=(o == 0),
                            stop=(o == NO - 1),
                        )
                    nc.vector.tensor_copy(vt_sb[:, jsl], ps[:])
                    for cc in range(QT // P):
                        c = j * (QT // P) + cc
                        tp = vt_ps.tile([P, P], bf16, tag="vtp")
                        nc.tensor.transpose(tp[:], vt_sb[:, bass.ts(c, P)], idn_sb[:])
                        nc.scalar.copy(v_sb[:, c, :], tp[:])

                # qT per head -> [128 dq, S] x4
                for h in range(HG):
                    for j in range(NJ):
                        jsl = bass.ts(j, QT)
                        ps = proj_ps.tile([P, QT], f32, tag="ps")
                        for o in range(NO):
                            nc.tensor.matmul(
                                ps[:],
                                wqt_sb[:, o, bass.ts(h, P)],
                                xt_sb[:, j, o, :],
                                start=(o == 0),
                                stop=(o == NO - 1),
                            )
                        nc.vector.tensor_copy(q_sb[:, h, jsl], ps[:])
                        rope(q_sb[:, h, :], jsl)

            # ---- phase 2: attention (block-outer) + per-block AllToAll +
            #      interleaved out-projection ----
            with (
                tc.tile_pool(name="attn_sb", bufs=3) as attn_sb,
                tc.tile_pool(name="osb", bufs=2) as osb_pool,
                tc.tile_pool(name="og", bufs=2) as og_pool,
                tc.tile_pool(name="fin", bufs=2) as fin,
                tc.tile_pool(name="ps_sc", bufs=2, space="PSUM") as ps_sc,
                tc.tile_pool(name="ps_acc", bufs=2, space="PSUM") as ps_acc,
                tc.tile_pool(name="ps_out", bufs=2, space="PSUM") as ps_out,
            ):

                def outproj(jblk):
                    # 128 output rows of 512-row block jblk, full 2048-feature
                    # contraction, results DMA'd per n-slice.
                    # Column order: whole blocks og[.., 64b+r]; the split last
                    # block og[.., 64*half + 32b + r], computed in two
                    # 64-row passes so pass 0 runs while the second half's
                    # a2a is still in flight.
                    pieces = [
                        (i, row0, w)
                        for i, (row0, w) in enumerate(blocks)
                        if 512 * jblk <= row0 < 512 * (jblk + 1)
                    ]
                    split = len(pieces) > 1
                    og_sb = og_pool.tile([P, NO, P], bf16, tag="og")
                    ot = fin.tile([P, D_MODEL], bf16, tag="ot")
                    for piece_idx, (i, _, w) in enumerate(pieces):
                        rb = w // N_CORES
                        src = a2a_out[i].rearrange(
                            "(b gfi p) r -> b p gfi r", b=B, p=P
                        )
                        for b in range(B):
                            c0 = 64 * piece_idx + rb * b if split else 64 * b
                            nc.sync.dma_start(og_sb[:, :, c0 : c0 + rb], src[b])
                    for n in range(D_MODEL // QT):
                        ps = ps_out.tile([P, QT], f32, tag="pout")
                        for c in range(NO):
                            nc.tensor.matmul(
                                ps[:],
                                og_sb[:, c, :],
                                wot_sb[:, c, bass.ts(n, QT)],
                                start=(c == 0),
                                stop=(c == NO - 1),
                            )
                        nsl = bass.ts(n, QT)
                        nc.scalar.copy(ot[:, nsl], ps[:])
                        nc.sync.dma_start(out_d[bass.ts(jblk, P), nsl], ot[:, nsl])

                # out-proj of block J is emitted two a2a's after J's own, so
                # each a2a gets ~2 attention blocks of slack before its data
                # is needed (the collectives re-sync all 8 cores, and the
                # slowest core can lag by ~25us)
                nb = len(blocks)
                ready_after = [max(0, i - 1) for i in range(nb)]
                ready_after[nb - 2] = NJ - 1
                ready_after[nb - 1] = NJ
                done_oproj = 0
                for i, (row0, w) in enumerate(blocks):
                    nk = (row0 + w) // P  # causal: k chunks 0..nk-1
                    w2 = 2 * w
                    o_sb = osb_pool.tile([P, HG, QT], bf16, tag="osb")
                    for h in range(HG):
                        po = ps_acc.tile([P, QT], f32, tag="po")
                        acc = attn_sb.tile([P, QT], bf16, tag="acc")
                        for c2 in range(nk // 2):
                            sc = ps_sc.tile([P, 2 * QT], f32, tag="sc")
                            pt = attn_sb.tile([P, 2 * QT], bf16, tag="pt")
                            for t in (0, 1):
                                c = 2 * c2 + t
                                nc.tensor.matmul(
                                    sc[:, t * w : (t + 1) * w],
                                    k_sb[:, bass.ts(c, P)],
                                    q_sb[:, h, row0 : row0 + w],
                                    start=True,
                                    stop=True,
                                )
                            nc.scalar.activation(
                                pt[:, 0:w2],
                                sc[:, 0:w2],
                                mybir.ActivationFunctionType.Exp,
                                scale=scale,
                            )
                            for t in (0, 1):
                                c = 2 * c2 + t
                                r = c - row0 // P
                                if 0 <= r < w // P:
                                    nc.vector.tensor_mul(
                                        pt[:, t * w : (t + 1) * w],
                                        pt[:, t * w : (t + 1) * w],
                                        msk_sb[:, r, 0:w],
                                    )
                            # softmax-denominator partials accumulate on DVE
                            for t in (0, 1):
                                c = 2 * c2 + t
                                if c == 0:
                                    nc.vector.tensor_copy(
                                        acc[:, 0:w], pt[:, 0:w]
                                    )
                                else:
                                    nc.vector.tensor_add(
                                        acc[:, 0:w],
                                        acc[:, 0:w],
                                        pt[:, t * w : (t + 1) * w],
                                    )
                            for t in (0, 1):
                                c = 2 * c2 + t
                                nc.tensor.matmul(
                                    po[:, 0:w],
                                    v_sb[:, c, :],
                                    pt[:, t * w : (t + 1) * w],
                                    start=(c == 0),
                                    stop=(c == nk - 1),
                                )
                        # partition-reduce acc + broadcast in one matmul; the
                        # rs tile shares the out-proj PSUM pool so po can be
                        # double-buffered within the 8-bank budget
                        rs = ps_out.tile([P, QT], f32, tag="pout")
                        nc.tensor.matmul(
                            rs[:, 0:w], ones_sb[:], acc[:, 0:w], start=True, stop=True
                        )
                        rcp = attn_sb.tile([P, QT], f32, tag="rcp")
                        nc.vector.reciprocal_approx_fast(rcp[:, 0:w], rs[:, 0:w])
                        nc.vector.tensor_mul(o_sb[:, h, 0:w], po[:, 0:w], rcp[:, 0:w])

                    # ship block i: a2a_in[i][(d*4+h)*128+p, r] = o_sb[p,h,rb*d+r]
                    in_v = a2a_in[i].rearrange("(d h p) r -> p h d r", h=HG, p=P)
                    for h in range(HG):
                        nc.sync.dma_start(
                            in_v[:, h],
                            o_sb[:, h, 0:w].rearrange("p (d r) -> p d r", d=N_CORES),
                        )
                    nc.gpsimd.collective_compute(
                        "AllToAll",
                        mybir.AluOpType.bypass,
                        replica_groups=ALL_CORES,
                        ins=[a2a_in[i].opt()],
                        outs=[a2a_out[i].opt()],
                    )
                    while done_oproj < ready_after[i]:
                        outproj(done_oproj)
                        done_oproj += 1
                while done_oproj < NJ:
                    outproj(done_oproj)
                    done_oproj += 1

    nc.compile()
    return nc


def host_prep(x, wq, wk, wv, wo, S):
    """Build the 8 per-core input maps (numpy, bf16)."""
    import ml_dtypes

    bf = ml_dtypes.bfloat16
    NJ = S // 512
    NO = D_MODEL // 128
    perm = np.concatenate(
        [np.arange(0, ROPE, 2), np.arange(1, ROPE, 2), np.arange(ROPE, HEAD_DIM)]
    )
    wq_p = wq.reshape(N_HEADS, HEAD_DIM, D_MODEL)[:, perm, :]
    wk_p = wk.reshape(N_KV, HEAD_DIM, D_MODEL)[:, perm, :]

    inv = THETA ** (-np.arange(0, ROPE, 2, dtype=np.float64) / ROPE)  # [32]
    t = np.arange(S, dtype=np.float64)
    ang = np.outer(inv, t)  # [32, S]
    cosT, sinT = np.cos(ang), np.sin(ang)
    cos = np.ascontiguousarray(np.concatenate([cosT, cosT], 0)).astype(bf)  # [64,S]
    sin = np.ascontiguousarray(np.concatenate([-sinT, sinT], 0)).astype(bf)  # [64,S]

    # causal masks for diagonal tiles: r = k_chunk - row0/128 in [0,4)
    kk = np.arange(128)[:, None]
    qq = np.arange(512)[None, :]
    masks = np.stack(
        [(128 * r + kk <= qq) for r in range(4)]
    ).astype(bf)  # [4,128,512]

    ident = np.eye(128, dtype=np.float32).astype(bf)

    def chunked(wT):  # [D, M] -> [128, NO, M] contiguous
        return np.ascontiguousarray(
            wT.reshape(NO, 128, wT.shape[1]).transpose(1, 0, 2)
        ).astype(bf)

    wot = chunked(wo.T.astype(np.float32))

    in_maps = []
    for c in range(N_CORES):
        b, g = divmod(c, 4)
        # xt[j, p, o, s] = x[b, 512j+s, 128o+p]
        xt = np.ascontiguousarray(
            x[b, :S].reshape(NJ, 512, NO, 128).transpose(0, 3, 2, 1)
        ).astype(bf)
        wqt = chunked(wq_p[HG * g : HG * (g + 1)].reshape(GD, D_MODEL).T)
        wkt = chunked(wk_p[g].T)
        wvt = chunked(wv[HEAD_DIM * g : HEAD_DIM * (g + 1)].T)
        in_maps.append(
            {
                "xt": xt,
                "wqt": wqt,
                "wkt": wkt,
                "wvt": wvt,
                "wot": wot,
                "cos": cos,
                "sin": sin,
                "masks": masks,
                "ident": ident,
            }
        )
    return in_maps


def core_rows(S, d):
    """Device out_d row index -> (batch, seq) for core d, in device order."""
    rows = []
    for jblk in range(S // 512):
        row0 = 512 * jblk
        if jblk < S // 512 - 1:
            for b in range(B):
                rows += [(b, row0 + 64 * d + r) for r in range(64)]
        else:
            for half in range(2):
                for b in range(B):
                    rows += [
                        (b, row0 + 256 * half + 32 * d + r) for r in range(32)
                    ]
    return rows


def run(x, wq, wk, wv, wo, S=None, trace=False):
    from concourse.bass_utils import run_bass_kernel_spmd

    if S is None:
        S = x.shape[1]
    if S not in _BUILD_CACHE:
        _BUILD_CACHE[S] = build_kernel(S)
    nc = _BUILD_CACHE[S]
    in_maps = host_prep(x, wq, wk, wv, wo, S)
    res = run_bass_kernel_spmd(nc, in_maps, core_ids=list(range(N_CORES)), trace=trace)
    out = np.empty((B, S, D_MODEL), np.float32)
    for d in range(N_CORES):
        o = np.asarray(res.results[d]["out"], dtype=np.float32)  # [S//4, D]
        rows = core_rows(S, d)
        bs = np.array([r[0] for r in rows])
        ss = np.array([r[1] for r in rows])
        out[bs, ss, :] = o
    return out, res


def kernel(x, wq, wk, wv, wo):
    x = np.asarray(x, np.float32)
    wq = np.asarray(wq, np.float32)
    wk = np.asarray(wk, np.float32)
    wv = np.asarray(wv, np.float32)
    wo = np.asarray(wo, np.float32)
    out, _ = run(x, wq, wk, wv, wo)
    return out
